# revision 9
# baseline (speedup 1.0000x reference)
"""Trainium2 Bass kernel for nn_E3nnMLPBlockS2Grid.

Data-parallel over batch (B=4096 -> 512 rows/core on 8 cores).

Math restructuring (validated to ~2e-6 abs err against the jax reference):
  - e3nn BatchNorm statistics are computed WITHOUT materializing y = x @ W1:
    per-l Gram matrices C_l = sum_{b,m} x_l[:,u,m] x_l[:,u',m]^T are built on
    the TensorEngine contracting the batch (partition) dim, so x stays in its
    natural [b, feat] layout.  fn[v] = diag(W1^T C W1) * inv^2 / (B d), which is
    linear in C, so per-core partials are AllReduce'd (a [128,8] f32 tile).
  - After the reduce, the whole Linear->BN->Linear front-end collapses to one
    vector per l:  A[l][u] = inv^2 * sum_v W1[l][u,v] * s[l][v] * W2[l][v]
    plus a scalar c0 for the l=0 centered/biased path.
  - z^T[36, b] is accumulated in PSUM via per-(l,m) matmuls with sparse
    one-column Psi matrices (lhsT = Psi_lm [128u, 36], rhs = x^T_lm [128u, b]).
  - S2Activation: grid^T = YTO^T z^T (79 g-chunks of 128), sigmoid on ScalarE
    directly out of PSUM, z2^T accumulated with lhsT = YFROM^T chunks; the
    final W3 expansion is a dense [36, 4608] matmul producing the output in
    natural [b, feat] layout.

x is column-permuted on the host so each (l,m) 128-column block is contiguous.
"""

import math

import numpy as np

import concourse.bass as bass
import concourse.tile as tile
from concourse import bacc, mybir
from concourse.bass_utils import run_bass_kernel_spmd

F32 = mybir.dt.float32

MUL = 128
LMAX = 5
G = 100
NCORES = 8
B = 4096
BS = B // NCORES            # 512 rows per core
FEAT = 4608
EPS = 1e-5
INV = 1.0 / math.sqrt(MUL)
DIMS = [2 * l + 1 for l in range(LMAX + 1)]
OFFS = np.cumsum([0] + [MUL * d for d in DIMS]).tolist()
HOFF = np.cumsum([0] + DIMS).tolist()
NPAIR = 36                  # total (l, m) pairs == hidden dim
L_OF = np.concatenate([np.full(d, l) for l, d in enumerate(DIMS)]).tolist()
GG = G * G                  # 10000
NGC = 79                    # ceil(10000 / 128) g-chunks
GPAD = NGC * 128            # 10112
BCHUNKS = BS // 128         # 4
SIG_GROUP = 3               # grid chunks per sigmoid call (3 PSUM banks)


# ---------------------------------------------------------------------------
# Host-side constants (S2 grid matrices etc. — identical math to the reference)
# ---------------------------------------------------------------------------

def _assoc_legendre(lmax, x):
    P = {(0, 0): np.ones_like(x)}
    s = np.sqrt(np.clip(1.0 - x * x, 0.0, None))
    for m in range(1, lmax + 1):
        P[(m, m)] = -(2 * m - 1) * s * P[(m - 1, m - 1)]
    for m in range(lmax):
        P[(m + 1, m)] = (2 * m + 1) * x * P[(m, m)]
    for m in range(lmax + 1):
        for l in range(m + 2, lmax + 1):
            P[(l, m)] = ((2 * l - 1) * x * P[(l - 1, m)] - (l + m - 1) * P[(l - 2, m)]) / (l - m)
    return P


def _build_s2_matrices():
    beta = (np.arange(G) + 0.5) * np.pi / G
    alpha = np.arange(G) * 2.0 * np.pi / G
    cb = np.cos(beta)
    P = _assoc_legendre(LMAX, cb)
    dim = (LMAX + 1) ** 2
    Y = np.zeros((dim, G, G))
    i = 0
    for l in range(LMAX + 1):
        for m in range(-l, l + 1):
            am = abs(m)
            N = math.sqrt((2 * l + 1) / (4 * math.pi) * math.factorial(l - am) / math.factorial(l + am))
            if m == 0:
                ang, c = np.ones(G), 1.0
            elif m > 0:
                ang, c = np.cos(m * alpha), math.sqrt(2.0)
            else:
                ang, c = np.sin(am * alpha), math.sqrt(2.0)
            Y[i] = c * N * P[(l, am)][:, None] * ang[None, :]
            i += 1
    V = np.polynomial.legendre.legvander(cb, G - 1).T
    e = np.zeros(G)
    e[0] = 2.0
    qw = np.linalg.solve(V, e)
    n_to = np.array([math.sqrt(4 * math.pi) * math.sqrt(2 * l + 1) / math.sqrt(LMAX + 1) for l in range(LMAX + 1)])
    lidx = np.concatenate([np.full(2 * l + 1, l, dtype=np.int64) for l in range(LMAX + 1)])
    Yto = (Y * n_to[lidx][:, None, None]).astype(np.float32)
    Yfrom = (Y * (1.0 / n_to)[lidx][:, None, None] * qw[None, :, None] * (2.0 * np.pi / G)).astype(np.float32)
    return Yto.reshape(dim, GG), Yfrom.reshape(dim, GG)


def _host_constants():
    """Data-independent constant arrays shipped as extra kernel inputs."""
    yto, yfrom = _build_s2_matrices()

    # YTO packed for lhsT use: chunk gc -> rows 64*(gc%2)..+36, cols 128*(gc//2)..+128
    yto_pad = np.zeros((36, GPAD), np.float32)
    yto_pad[:, :GG] = yto
    yto2 = np.zeros((100, 40 * 128), np.float32)
    for gc in range(NGC):
        h, cblk = gc % 2, gc // 2
        yto2[64 * h:64 * h + 36, 128 * cblk:128 * (cblk + 1)] = yto_pad[:, 128 * gc:128 * (gc + 1)]

    # YFROM^T packed: chunk gc -> [128(g within chunk), 36] at cols 36*gc (zero-padded g)
    yfromt = np.zeros((128, NGC * 36), np.float32)
    for gc in range(NGC):
        lo, hi = 128 * gc, min(128 * (gc + 1), GG)
        yfromt[: hi - lo, 36 * gc:36 * (gc + 1)] = yfrom[:, lo:hi].T

    ident = np.eye(128, dtype=np.float32)
    onescol = np.ones((128, 1), np.float32)
    onesrow = np.ones((1, 128), np.float32)
    # per-l scaling for the fn column-sum matmul: inv^2 / (B * d)
    fnsc = np.zeros((128, 6), np.float32)
    for l in range(6):
        fnsc[:, l] = INV * INV / (B * DIMS[l])
    return {
        "yto2": yto2,
        "yfromt": yfromt,
        "ident": ident,
        "onescol": onescol,
        "onesrow": onesrow,
        "fnsc": fnsc,
    }


def _host_weights(W1, W2, W3, bn_w, bn_b):
    """Weight-derived arrays (runtime inputs, transformed on host)."""
    W1 = np.asarray(W1, np.float32)
    W2 = np.asarray(W2, np.float32)
    W3 = np.asarray(W3, np.float32)
    bn_w = np.asarray(bn_w, np.float32)
    bn_b = np.asarray(bn_b, np.float32)

    w1 = np.zeros((128, 768), np.float32)      # [u, (l v)]
    w1t = np.zeros((128, 768), np.float32)     # [v, (l u)]
    for l in range(6):
        w1[:, 128 * l:128 * (l + 1)] = W1[l]
        w1t[:, 128 * l:128 * (l + 1)] = W1[l].T
    bnwt = bn_w.T.copy()                       # [128(v), 6]
    w2s = (W2.T * (INV * INV)).astype(np.float32)  # [128(v), 6]
    bnbw = (bn_b * W2[0] * INV).reshape(128, 1).astype(np.float32)

    # Expansion matrix E packed like yto2: chunk fc -> rows 64*(fc%2)..+36,
    # cols 512*(fc//2)..+512.  E[i, f] = W3[l(f)][v(f)] when i == HOFF_l + m(f).
    E = np.zeros((36, FEAT), np.float32)
    for l in range(6):
        d = DIMS[l]
        for m in range(d):
            E[HOFF[l] + m, OFFS[l] + m:OFFS[l + 1]:d] = W3[l]
    e2 = np.zeros((100, 5 * 512), np.float32)
    for fc in range(9):
        h, cblk = fc % 2, fc // 2
        e2[64 * h:64 * h + 36, 512 * cblk:512 * (cblk + 1)] = E[:, 512 * fc:512 * (fc + 1)]
    return {
        "w1": w1,
        "w1t": w1t,
        "bnwt": bnwt,
        "w2s": w2s,
        "bnbw": bnbw,
        "e2": e2,
    }


def _permute_x(x):
    """Column-permute x so each (l, m) block of 128 u's is contiguous:
    new col index for pair p=(l,m): 128*p + u (old: OFFS[l] + u*d + m)."""
    perm = np.zeros(FEAT, np.int64)
    p = 0
    for l in range(6):
        d = DIMS[l]
        for m in range(d):
            perm[128 * p:128 * (p + 1)] = OFFS[l] + m + d * np.arange(128)
            p += 1
    return np.ascontiguousarray(np.asarray(x, np.float32)[:, perm])


# ---------------------------------------------------------------------------
# The Bass program (SPMD, one NeuronCore shown; run on 8)
# ---------------------------------------------------------------------------

def build_nc():
    # Bacc (not raw Bass): its compile() pipeline splits multi-semaphore waits
    # (TRN2 matmuls support a single sync wait) via generate_event_semaphores.
    nc = bacc.Bacc(None, num_devices=NCORES)

    xp = nc.dram_tensor("xp", [BS, FEAT], F32, kind="ExternalInput")
    d_w1 = nc.dram_tensor("w1", [128, 768], F32, kind="ExternalInput")
    d_w1t = nc.dram_tensor("w1t", [128, 768], F32, kind="ExternalInput")
    d_bnwt = nc.dram_tensor("bnwt", [128, 6], F32, kind="ExternalInput")
    d_w2s = nc.dram_tensor("w2s", [128, 6], F32, kind="ExternalInput")
    d_bnbw = nc.dram_tensor("bnbw", [128, 1], F32, kind="ExternalInput")
    d_e2 = nc.dram_tensor("e2", [100, 2560], F32, kind="ExternalInput")
    d_yto2 = nc.dram_tensor("yto2", [100, 5120], F32, kind="ExternalInput")
    d_yfromt = nc.dram_tensor("yfromt", [128, NGC * 36], F32, kind="ExternalInput")
    d_ident = nc.dram_tensor("ident", [128, 128], F32, kind="ExternalInput")
    d_onescol = nc.dram_tensor("onescol", [128, 1], F32, kind="ExternalInput")
    d_onesrow = nc.dram_tensor("onesrow", [1, 128], F32, kind="ExternalInput")
    d_fnsc = nc.dram_tensor("fnsc", [128, 6], F32, kind="ExternalInput")

    out = nc.dram_tensor("out", [BS, FEAT], F32, kind="ExternalOutput")

    with tile.TileContext(nc) as tc:
        with (
            tc.tile_pool(name="consts", bufs=1) as consts,
            tc.tile_pool(name="work", bufs=1) as work,
            tc.tile_pool(name="xin", bufs=2) as xin,
            tc.tile_pool(name="xt", bufs=BCHUNKS) as xtp,
        ):
            # ---- load constants / weights ----
            t_w1 = consts.tile([128, 768], F32)
            t_w1t = consts.tile([128, 768], F32)
            t_bnwt = consts.tile([128, 6], F32)
            t_w2s = consts.tile([128, 6], F32)
            t_bnbw = consts.tile([128, 1], F32)
            t_e2 = consts.tile([100, 2560], F32)
            t_yto2 = consts.tile([100, 5120], F32)
            t_yfromt = consts.tile([128, NGC * 36], F32)
            t_ident = consts.tile([128, 128], F32)
            t_onescol = consts.tile([128, 1], F32)
            t_onesrow = consts.tile([1, 128], F32)
            t_fnsc = consts.tile([128, 6], F32)
            for t, d in [
                (t_w1, d_w1), (t_w1t, d_w1t), (t_bnwt, d_bnwt), (t_w2s, d_w2s),
                (t_bnbw, d_bnbw), (t_e2, d_e2), (t_yto2, d_yto2),
                (t_yfromt, d_yfromt), (t_ident, d_ident), (t_onescol, d_onescol),
                (t_onesrow, d_onesrow), (t_fnsc, d_fnsc),
            ]:
                nc.sync.dma_start(out=t[:], in_=d[:])

            # ---- persistent SBUF work tiles ----
            t_C = work.tile([128, 768], F32)      # Gram accumulators [u, (l u')]
            t_S0 = work.tile([128, 1], F32)       # sum_b x_0[b, u]
            t_stats = work.tile([128, 8], F32)    # AR payload
            t_statsg = work.tile([128, 8], F32)   # AR result
            t_eps = work.tile([128, 1], F32)
            t_P = work.tile([128, 128], F32)
            t_A = work.tile([128, 6], F32)
            t_psi = work.tile([128, NPAIR * 36], F32)
            t_c0row = work.tile([1, 36], F32)
            # z^T / z2^T live at partitions 0:36 and are replicated to 64:100
            # (matmul requires lhsT/rhs to share a base partition, and the
            # yto2/e2 constants are packed two chunks per 128 partitions).
            t_zts = work.tile([100, BS], F32)     # z^T
            t_z2s = work.tile([100, BS], F32)     # z2^T
            t_tmp1 = work.tile([128, 6], F32)
            t_tmp2 = work.tile([128, 1], F32)

            nc.vector.memset(t_C, 0.0)
            nc.vector.memset(t_S0, 0.0)
            nc.vector.memset(t_stats, 0.0)
            nc.vector.memset(t_eps, EPS)
            nc.gpsimd.memset(t_psi, 0.0)
            nc.gpsimd.memset(t_c0row, 0.0)

            # =========== Phase A: load x, transposes, Gram partials ==========
            xts = []
            with (
                tc.tile_pool(name="ps_tr", bufs=3, space="PSUM") as pst,
                tc.tile_pool(name="ps_gram", bufs=2, space="PSUM") as psg,
                tc.tile_pool(name="ps_s0", bufs=2, space="PSUM") as pss,
            ):
                for c in range(BCHUNKS):
                    x_c = xin.tile([128, FEAT], F32)
                    nc.sync.dma_start(out=x_c[:], in_=xp[128 * c:128 * (c + 1), :])

                    # PE transposes: x^T per (l, m) pair; copies split DVE/ACT
                    xt_c = xtp.tile([128, FEAT], F32, tag="xt")
                    for p in range(NPAIR):
                        tp = pst.tile([128, 128], F32)
                        nc.tensor.matmul(tp, x_c[:, 128 * p:128 * (p + 1)],
                                         t_ident[:], start=True, stop=True)
                        dst = xt_c[:, 128 * p:128 * (p + 1)]
                        if p % 2 == 0:
                            nc.vector.tensor_copy(dst, tp)
                        else:
                            nc.scalar.copy(out=dst, in_=tp)
                    xts.append(xt_c)

                    # Gram: per l accumulate over m in PSUM, then add into SBUF
                    for l in range(6):
                        d = DIMS[l]
                        cg = psg.tile([128, 128], F32)
                        for m in range(d):
                            p = HOFF[l] + m
                            sl = x_c[:, 128 * p:128 * (p + 1)]
                            nc.tensor.matmul(cg, sl, sl,
                                             start=(m == 0), stop=(m == d - 1))
                        nc.vector.tensor_add(t_C[:, 128 * l:128 * (l + 1)],
                                             t_C[:, 128 * l:128 * (l + 1)], cg)
                    # S0 partial (l=0 block is pair p=0)
                    s0 = pss.tile([128, 1], F32)
                    nc.tensor.matmul(s0, x_c[:, 0:128], t_onescol[:],
                                     start=True, stop=True)
                    nc.vector.tensor_add(t_S0, t_S0, s0)

            # =========== Phase B: stats extraction + AllReduce ==========
            with tc.tile_pool(name="ps_small", bufs=4, space="PSUM") as pssm:
                fn_ps = pssm.tile([128, 6], F32, tag="persist")
                for l in range(6):
                    t_ps = pssm.tile([128, 128], F32, tag="T")
                    nc.tensor.matmul(t_ps, t_C[:, 128 * l:128 * (l + 1)],
                                     t_w1[:, 128 * l:128 * (l + 1)],
                                     start=True, stop=True)
                    nc.vector.tensor_mul(t_P, t_w1[:, 128 * l:128 * (l + 1)], t_ps)
                    nc.tensor.matmul(fn_ps[:, l:l + 1], t_P[:],
                                     t_fnsc[:, l:l + 1], start=True, stop=True)
                ybar_ps = pssm.tile([128, 1], F32, tag="T")
                nc.tensor.matmul(ybar_ps, t_w1[:, 0:128], t_S0[:],
                                 start=True, stop=True)
                nc.vector.tensor_copy(t_stats[:, 0:6], fn_ps)
                nc.scalar.mul(out=t_stats[:, 6:7], in_=ybar_ps, mul=INV / B)

                with tc.tile_pool(name="dram", bufs=1, space="DRAM") as dpool:
                    cc_in = dpool.tile([128, 8], F32)
                    cc_out = dpool.tile([128, 8], F32)
                    nc.gpsimd.dma_start(out=cc_in[:], in_=t_stats[:])
                    nc.gpsimd.collective_compute(
                        "AllReduce",
                        mybir.AluOpType.add,
                        replica_groups=[list(range(NCORES))],
                        ins=[cc_in[:].opt()],
                        outs=[cc_out[:].opt()],
                    )
                    nc.gpsimd.dma_start(out=t_statsg[:], in_=cc_out[:])

                # ---- post-reduce: s, q, A, c0, Psi ----
                # fn0 -= ybar^2 ; s = bn_w / sqrt(fn + eps) ; q = s * W2^T * inv^2
                nc.vector.tensor_mul(t_tmp2, t_statsg[:, 6:7], t_statsg[:, 6:7])
                nc.vector.tensor_sub(t_statsg[:, 0:1], t_statsg[:, 0:1], t_tmp2)
                nc.scalar.activation(out=t_tmp1, in_=t_statsg[:, 0:6],
                                     func=mybir.ActivationFunctionType.Sqrt,
                                     bias=t_eps, scale=1.0)
                nc.vector.reciprocal(t_tmp1, t_tmp1)
                nc.vector.tensor_mul(t_tmp1, t_tmp1, t_bnwt)   # s [v, l]
                nc.vector.tensor_mul(t_tmp1, t_tmp1, t_w2s)    # q [v, l]

                a_ps = pssm.tile([128, 6], F32, tag="T")
                for l in range(6):
                    nc.tensor.matmul(a_ps[:, l:l + 1],
                                     t_w1t[:, 128 * l:128 * (l + 1)],
                                     t_tmp1[:, l:l + 1], start=True, stop=True)
                nc.vector.tensor_copy(t_A, a_ps)

                # c0 = sum_v (bnbw - ybar * q0 * sqrt(128))
                nc.vector.tensor_mul(t_tmp2, t_statsg[:, 6:7], t_tmp1[:, 0:1])
                nc.scalar.mul(out=t_tmp2, in_=t_tmp2, mul=math.sqrt(float(MUL)))
                nc.vector.tensor_sub(t_tmp2, t_bnbw, t_tmp2)
                c0_ps = pssm.tile([1, 1], F32, tag="T")
                nc.tensor.matmul(c0_ps, t_tmp2[:], t_onescol[:],
                                 start=True, stop=True)
                nc.vector.tensor_copy(t_c0row[0:1, 0:1], c0_ps)

                # Psi: column i of pair-p tile gets A[:, l(p)] (i == p)
                for p in range(NPAIR):
                    dst = t_psi[:, 36 * p + p:36 * p + p + 1]
                    src = t_A[:, L_OF[p]:L_OF[p] + 1]
                    if p % 2 == 0:
                        nc.vector.tensor_copy(dst, src)
                    else:
                        nc.scalar.copy(out=dst, in_=src)

            # =========== Phase C: z^T ==========
            with tc.tile_pool(name="ps_z", bufs=2, space="PSUM") as psz:
                for c in range(BCHUNKS):
                    zt_ps = psz.tile([36, 128], F32)
                    for p in range(NPAIR):
                        nc.tensor.matmul(zt_ps, t_psi[:, 36 * p:36 * (p + 1)],
                                         xts[c][:, 128 * p:128 * (p + 1)],
                                         start=(p == 0), stop=False)
                    nc.tensor.matmul(zt_ps, t_c0row[:], t_onesrow[:],
                                     start=False, stop=True)
                    nc.vector.tensor_copy(t_zts[0:36, 128 * c:128 * (c + 1)], zt_ps)
                # replicate z^T to partitions 64:100 for the odd-half chunks
                nc.sync.dma_start(out=t_zts[64:100, :], in_=t_zts[0:36, :])

            # =========== Phase D: S2 grid -> sigmoid -> z2 ==========
            with (
                tc.tile_pool(name="ps_grid", bufs=2, space="PSUM") as psgr,
                tc.tile_pool(name="ps_z2", bufs=1, space="PSUM") as psz2,
                tc.tile_pool(name="sg", bufs=2) as sgp,
            ):
                z2_ps = psz2.tile([36, BS], F32)
                groups = [list(range(s, min(s + SIG_GROUP, NGC)))
                          for s in range(0, NGC, SIG_GROUP)]
                for grp in groups:
                    nj = len(grp)
                    gr_ps = psgr.tile([128, SIG_GROUP, BS], F32, tag="grid")
                    sg = sgp.tile([128, SIG_GROUP, BS], F32, tag="sg")
                    for j, gc in enumerate(grp):
                        h, cblk = gc % 2, gc // 2
                        nc.tensor.matmul(
                            gr_ps[:, j, :],
                            t_yto2[64 * h:64 * h + 36, 128 * cblk:128 * (cblk + 1)],
                            t_zts[64 * h:64 * h + 36, :], start=True, stop=True)
                    nc.scalar.activation(out=sg[:, 0:nj, :], in_=gr_ps[:, 0:nj, :],
                                         func=mybir.ActivationFunctionType.Sigmoid)
                    for j, gc in enumerate(grp):
                        nc.tensor.matmul(z2_ps, t_yfromt[:, 36 * gc:36 * (gc + 1)],
                                         sg[:, j, :],
                                         start=(gc == 0), stop=(gc == NGC - 1))
                nc.vector.tensor_copy(t_z2s[0:36, :], z2_ps)
                nc.sync.dma_start(out=t_z2s[64:100, :], in_=t_z2s[0:36, :])

            # =========== Phase E: W3 expansion + output ==========
            with (
                tc.tile_pool(name="ps_out", bufs=3, space="PSUM") as pso,
                tc.tile_pool(name="osb", bufs=2) as osbp,
            ):
                for c in range(BCHUNKS):
                    osb = osbp.tile([128, FEAT], F32, tag="osb")
                    for fc in range(9):
                        h, cblk = fc % 2, fc // 2
                        o_ps = pso.tile([128, 512], F32)
                        nc.tensor.matmul(
                            o_ps, t_z2s[64 * h:64 * h + 36, 128 * c:128 * (c + 1)],
                            t_e2[64 * h:64 * h + 36, 512 * cblk:512 * (cblk + 1)],
                            start=True, stop=True)
                        dst = osb[:, 512 * fc:512 * (fc + 1)]
                        if fc % 2 == 0:
                            nc.vector.tensor_copy(dst, o_ps)
                        else:
                            nc.scalar.copy(out=dst, in_=o_ps)
                    nc.sync.dma_start(out=out[128 * c:128 * (c + 1), :], in_=osb[:])

    nc.compile()
    return nc


# ---------------------------------------------------------------------------
# Public entry point
# ---------------------------------------------------------------------------

_CACHE = {}


def _get_nc():
    if "nc" not in _CACHE:
        _CACHE["nc"] = build_nc()
    return _CACHE["nc"]


def make_in_maps(x, W1, W2, W3, bn_w, bn_b):
    consts = _host_constants()
    weights = _host_weights(W1, W2, W3, bn_w, bn_b)
    xp = _permute_x(x)
    shared = {**consts, **weights}
    in_maps = []
    for c in range(NCORES):
        m = dict(shared)
        m["xp"] = np.ascontiguousarray(xp[BS * c:BS * (c + 1)])
        in_maps.append(m)
    return in_maps


def kernel(x, W1, W2, W3, bn_w, bn_b, trace=False, **run_kwargs):
    nc = _get_nc()
    in_maps = make_in_maps(x, W1, W2, W3, bn_w, bn_b)
    res = run_bass_kernel_spmd(nc, in_maps, list(range(NCORES)),
                               trace=trace, **run_kwargs)
    out = np.concatenate([res.results[c]["out"] for c in range(NCORES)], axis=0)
    kernel.last_results = res
    return out.astype(np.float32)


# revision 14
# speedup vs baseline: 1.1184x; 1.1184x over previous
"""Trainium2 Bass kernel for nn_E3nnMLPBlockS2Grid.

Data-parallel over batch (B=4096 -> 512 rows/core on 8 cores).

Math restructuring (validated to ~2e-6 abs err against the jax reference):
  - e3nn BatchNorm statistics are computed WITHOUT materializing y = x @ W1:
    per-l Gram matrices C_l = sum_{b,m} x_l[:,u,m] x_l[:,u',m]^T are built on
    the TensorEngine contracting the batch (partition) dim, so x stays in its
    natural [b, feat] layout.  fn[v] = diag(W1^T C W1) * inv^2 / (B d), which is
    linear in C, so per-core partials are AllReduce'd (a [128,8] f32 tile).
  - After the reduce, the whole Linear->BN->Linear front-end collapses to one
    vector per l:  A[l][u] = inv^2 * sum_v W1[l][u,v] * s[l][v] * W2[l][v]
    plus a scalar c0 for the l=0 centered/biased path.
  - z^T[36, b] is accumulated in PSUM via per-(l,m) matmuls with sparse
    one-column Psi matrices (lhsT = Psi_lm [128u, 36], rhs = x^T_lm [128u, b]).
    The x^T tiles are built by PE transpose-matmuls against identity, scheduled
    inside the AllReduce window (x is read a second time from HBM for this so
    SBUF holds only a 2-chunk window of natural-layout x).
  - S2Activation: grid^T = YTO^T z^T (79 g-chunks of 128), sigmoid on ScalarE
    directly out of PSUM, z2^T accumulated with lhsT = YFROM^T chunks; the
    final W3 expansion is a dense [36, 4608] matmul producing the output in
    natural [b, feat] layout.

x is column-permuted on the host so each (l,m) 128-column block is contiguous.
COMPUTE_DT selects the streaming dtype (bfloat16 halves PE col time + DMA
bytes; all matmul accumulation and the statistics path stay fp32).
"""

import math

import numpy as np
import ml_dtypes

import concourse.bass as bass
import concourse.tile as tile
from concourse import bacc, mybir
from concourse.bass_utils import run_bass_kernel_spmd

F32 = mybir.dt.float32
# Streaming dtype for x / S2 constants. bfloat16 halves PE-column time and DMA
# bytes but costs ~2e-2 max scale-relative error end-to-end (measured); fp32
# keeps it at ~1e-5. Correctness margin wins: fp32.
COMPUTE_DT = mybir.dt.float32
NP_COMPUTE = np.float32

MUL = 128
LMAX = 5
G = 100
NCORES = 8
B = 4096
BS = B // NCORES            # 512 rows per core
FEAT = 4608
EPS = 1e-5
INV = 1.0 / math.sqrt(MUL)
DIMS = [2 * l + 1 for l in range(LMAX + 1)]
OFFS = np.cumsum([0] + [MUL * d for d in DIMS]).tolist()
HOFF = np.cumsum([0] + DIMS).tolist()
NPAIR = 36                  # total (l, m) pairs == hidden dim
L_OF = np.concatenate([np.full(d, l) for l, d in enumerate(DIMS)]).tolist()
GG = G * G                  # 10000
NGC = 79                    # ceil(10000 / 128) g-chunks
BCHUNKS = BS // 128         # 4
SIG_GROUP = 3               # grid chunks per sigmoid call (3 PSUM banks)


# ---------------------------------------------------------------------------
# Host-side constants (S2 grid matrices etc. — identical math to the reference)
# ---------------------------------------------------------------------------

def _assoc_legendre(lmax, x):
    P = {(0, 0): np.ones_like(x)}
    s = np.sqrt(np.clip(1.0 - x * x, 0.0, None))
    for m in range(1, lmax + 1):
        P[(m, m)] = -(2 * m - 1) * s * P[(m - 1, m - 1)]
    for m in range(lmax):
        P[(m + 1, m)] = (2 * m + 1) * x * P[(m, m)]
    for m in range(lmax + 1):
        for l in range(m + 2, lmax + 1):
            P[(l, m)] = ((2 * l - 1) * x * P[(l - 1, m)] - (l + m - 1) * P[(l - 2, m)]) / (l - m)
    return P


def _build_s2_matrices():
    beta = (np.arange(G) + 0.5) * np.pi / G
    alpha = np.arange(G) * 2.0 * np.pi / G
    cb = np.cos(beta)
    P = _assoc_legendre(LMAX, cb)
    dim = (LMAX + 1) ** 2
    Y = np.zeros((dim, G, G))
    i = 0
    for l in range(LMAX + 1):
        for m in range(-l, l + 1):
            am = abs(m)
            N = math.sqrt((2 * l + 1) / (4 * math.pi) * math.factorial(l - am) / math.factorial(l + am))
            if m == 0:
                ang, c = np.ones(G), 1.0
            elif m > 0:
                ang, c = np.cos(m * alpha), math.sqrt(2.0)
            else:
                ang, c = np.sin(am * alpha), math.sqrt(2.0)
            Y[i] = c * N * P[(l, am)][:, None] * ang[None, :]
            i += 1
    V = np.polynomial.legendre.legvander(cb, G - 1).T
    e = np.zeros(G)
    e[0] = 2.0
    qw = np.linalg.solve(V, e)
    n_to = np.array([math.sqrt(4 * math.pi) * math.sqrt(2 * l + 1) / math.sqrt(LMAX + 1) for l in range(LMAX + 1)])
    lidx = np.concatenate([np.full(2 * l + 1, l, dtype=np.int64) for l in range(LMAX + 1)])
    Yto = (Y * n_to[lidx][:, None, None]).astype(np.float32)
    Yfrom = (Y * (1.0 / n_to)[lidx][:, None, None] * qw[None, :, None] * (2.0 * np.pi / G)).astype(np.float32)
    return Yto.reshape(dim, GG), Yfrom.reshape(dim, GG)


def _host_constants():
    """Data-independent constant arrays shipped as extra kernel inputs."""
    yto, yfrom = _build_s2_matrices()

    # YTO packed for lhsT use: chunk gc -> rows 64*(gc%2)..+36, cols 128*(gc//2)..+128
    yto_pad = np.zeros((36, NGC * 128), np.float32)
    yto_pad[:, :GG] = yto
    yto2 = np.zeros((100, 40 * 128), np.float32)
    for gc in range(NGC):
        h, cblk = gc % 2, gc // 2
        yto2[64 * h:64 * h + 36, 128 * cblk:128 * (cblk + 1)] = yto_pad[:, 128 * gc:128 * (gc + 1)]

    # YFROM^T packed: chunk gc -> [128(g within chunk), 36] at cols 36*gc (zero-padded g)
    yfromt = np.zeros((128, NGC * 36), np.float32)
    for gc in range(NGC):
        lo, hi = 128 * gc, min(128 * (gc + 1), GG)
        yfromt[: hi - lo, 36 * gc:36 * (gc + 1)] = yfrom[:, lo:hi].T

    ident = np.eye(128, dtype=np.float32)
    onescol = np.ones((128, 1), np.float32)
    onesrow = np.ones((1, 128), np.float32)
    # per-l scaling for the fn column-sum matmul: inv^2 / (B * d)
    fnsc = np.zeros((128, 6), np.float32)
    for l in range(6):
        fnsc[:, l] = INV * INV / (B * DIMS[l])
    return {
        "yto2": yto2.astype(NP_COMPUTE),
        "yfromt": yfromt.astype(NP_COMPUTE),
        "ident": ident.astype(NP_COMPUTE),
        "onescol": onescol.astype(NP_COMPUTE),
        "onesrow": onesrow.astype(NP_COMPUTE),
        "fnsc": fnsc,
    }


def _host_weights(W1, W2, W3, bn_w, bn_b):
    """Weight-derived arrays (runtime inputs, transformed on host)."""
    W1 = np.asarray(W1, np.float32)
    W2 = np.asarray(W2, np.float32)
    W3 = np.asarray(W3, np.float32)
    bn_w = np.asarray(bn_w, np.float32)
    bn_b = np.asarray(bn_b, np.float32)

    w1 = np.zeros((128, 768), np.float32)      # [u, (l v)]
    w1t = np.zeros((128, 768), np.float32)     # [v, (l u)]
    for l in range(6):
        w1[:, 128 * l:128 * (l + 1)] = W1[l]
        w1t[:, 128 * l:128 * (l + 1)] = W1[l].T
    bnwt = bn_w.T.copy()                       # [128(v), 6]
    w2s = (W2.T * (INV * INV)).astype(np.float32)  # [128(v), 6]
    bnbw = (bn_b * W2[0] * INV).reshape(128, 1).astype(np.float32)

    # Expansion matrix E packed like yto2: chunk fc -> rows 64*(fc%2)..+36,
    # cols 512*(fc//2)..+512.  E[i, f] = W3[l(f)][v(f)] when i == HOFF_l + m(f).
    E = np.zeros((36, FEAT), np.float32)
    for l in range(6):
        d = DIMS[l]
        for m in range(d):
            E[HOFF[l] + m, OFFS[l] + m:OFFS[l + 1]:d] = W3[l]
    e2 = np.zeros((100, 5 * 512), np.float32)
    for fc in range(9):
        h, cblk = fc % 2, fc // 2
        e2[64 * h:64 * h + 36, 512 * cblk:512 * (cblk + 1)] = E[:, 512 * fc:512 * (fc + 1)]
    return {
        "w1": w1,
        "w1t": w1t,
        "bnwt": bnwt,
        "w2s": w2s,
        "bnbw": bnbw,
        "e2": e2.astype(NP_COMPUTE),
    }


def _permute_x(x):
    """Column-permute x so each (l, m) block of 128 u's is contiguous:
    new col index for pair p=(l,m): 128*p + u (old: OFFS[l] + u*d + m)."""
    perm = np.zeros(FEAT, np.int64)
    p = 0
    for l in range(6):
        d = DIMS[l]
        for m in range(d):
            perm[128 * p:128 * (p + 1)] = OFFS[l] + m + d * np.arange(128)
            p += 1
    return np.ascontiguousarray(np.asarray(x, np.float32)[:, perm].astype(NP_COMPUTE))


# ---------------------------------------------------------------------------
# The Bass program (SPMD, one NeuronCore shown; run on 8)
# ---------------------------------------------------------------------------

def build_nc():
    # Bacc (not raw Bass): its compile() pipeline splits multi-semaphore waits
    # (TRN2 matmuls support a single sync wait) via generate_event_semaphores.
    nc = bacc.Bacc(None, num_devices=NCORES)
    CD = COMPUTE_DT

    xp = nc.dram_tensor("xp", [BS, FEAT], CD, kind="ExternalInput")
    d_w1 = nc.dram_tensor("w1", [128, 768], F32, kind="ExternalInput")
    d_w1t = nc.dram_tensor("w1t", [128, 768], F32, kind="ExternalInput")
    d_bnwt = nc.dram_tensor("bnwt", [128, 6], F32, kind="ExternalInput")
    d_w2s = nc.dram_tensor("w2s", [128, 6], F32, kind="ExternalInput")
    d_bnbw = nc.dram_tensor("bnbw", [128, 1], F32, kind="ExternalInput")
    d_e2 = nc.dram_tensor("e2", [100, 2560], CD, kind="ExternalInput")
    d_yto2 = nc.dram_tensor("yto2", [100, 5120], CD, kind="ExternalInput")
    d_yfromt = nc.dram_tensor("yfromt", [128, NGC * 36], CD, kind="ExternalInput")
    d_ident = nc.dram_tensor("ident", [128, 128], CD, kind="ExternalInput")
    d_onescol = nc.dram_tensor("onescol", [128, 1], CD, kind="ExternalInput")
    d_onesrow = nc.dram_tensor("onesrow", [1, 128], CD, kind="ExternalInput")
    d_fnsc = nc.dram_tensor("fnsc", [128, 6], F32, kind="ExternalInput")

    out = nc.dram_tensor("out", [BS, FEAT], F32, kind="ExternalOutput")

    with tile.TileContext(nc) as tc:
        with (
            tc.tile_pool(name="consts", bufs=1) as consts,
            tc.tile_pool(name="work", bufs=1) as work,
            tc.tile_pool(name="xin", bufs=2) as xin,
            tc.tile_pool(name="xt", bufs=BCHUNKS) as xtp,
        ):
            # ---- x chunk 0 + phase-A constants first (critical path) ----
            t_ident = consts.tile([128, 128], CD)
            t_onescol = consts.tile([128, 1], CD)
            x_tiles = []
            x0 = xin.tile([128, FEAT], CD, tag="x")
            nc.sync.dma_start(out=x0[:], in_=xp[0:128, :])
            nc.scalar.dma_start(out=t_onescol[:], in_=d_onescol[:])
            x_tiles.append(x0)
            for c in range(1, BCHUNKS):
                x_c = xin.tile([128, FEAT], CD, tag="x")
                nc.sync.dma_start(out=x_c[:], in_=xp[128 * c:128 * (c + 1), :])
                x_tiles.append(x_c)

            # ---- remaining constants (scalar-engine HWDGE queue) ----
            t_w1 = consts.tile([128, 768], F32)
            t_w1t = consts.tile([128, 768], F32)
            t_bnwt = consts.tile([128, 6], F32)
            t_w2s = consts.tile([128, 6], F32)
            t_bnbw = consts.tile([128, 1], F32)
            t_fnsc = consts.tile([128, 6], F32)
            t_yto2 = consts.tile([100, 5120], CD)
            t_yfromt = consts.tile([128, NGC * 36], CD)
            t_e2 = consts.tile([100, 2560], CD)
            t_onesrow = consts.tile([1, 128], CD)
            for t, d in [
                (t_w1, d_w1), (t_fnsc, d_fnsc), (t_ident, d_ident),
                (t_onesrow, d_onesrow),
                (t_w1t, d_w1t), (t_bnwt, d_bnwt), (t_w2s, d_w2s),
                (t_bnbw, d_bnbw), (t_yto2, d_yto2), (t_yfromt, d_yfromt),
                (t_e2, d_e2),
            ]:
                nc.scalar.dma_start(out=t[:], in_=d[:])

            # ---- persistent SBUF work tiles ----
            t_C = work.tile([128, 768], F32)      # Gram accumulators [u, (l u')]
            t_S0 = work.tile([128, 1], F32)       # sum_b x_0[b, u]
            t_stats = work.tile([128, 8], F32)    # AR payload
            t_statsg = work.tile([128, 8], F32)   # AR result
            t_eps = work.tile([128, 1], F32)
            t_P = work.tile([128, 128], F32)
            t_A = work.tile([128, 6], F32)
            t_psi = work.tile([128, NPAIR * 36], CD)
            t_c0row = work.tile([1, 36], CD)
            # z^T / z2^T live at partitions 0:36 and are replicated to 64:100
            # (matmul requires lhsT/rhs to share a base partition, and the
            # yto2/e2 constants are packed two chunks per 128 partitions).
            t_zts = work.tile([100, BS], CD)      # z^T
            t_z2s = work.tile([100, BS], CD)      # z2^T
            t_tmp1 = work.tile([128, 6], F32)
            t_tmp2 = work.tile([128, 1], F32)

            nc.vector.memset(t_C, 0.0)
            nc.vector.memset(t_S0, 0.0)
            nc.vector.memset(t_stats, 0.0)
            nc.vector.memset(t_eps, EPS)
            nc.gpsimd.memset(t_psi, 0.0)
            nc.gpsimd.memset(t_c0row, 0.0)

            # =========== Phase A: Gram + S0 partials ==========
            with (
                tc.tile_pool(name="ps_gram", bufs=2, space="PSUM") as psg,
                tc.tile_pool(name="ps_s0", bufs=2, space="PSUM") as pss,
            ):
                for c in range(BCHUNKS):
                    x_c = x_tiles[c]
                    for l in range(6):
                        d = DIMS[l]
                        cg = psg.tile([128, 128], F32)
                        for m in range(d):
                            p = HOFF[l] + m
                            sl = x_c[:, 128 * p:128 * (p + 1)]
                            nc.tensor.matmul(cg, sl, sl,
                                             start=(m == 0), stop=(m == d - 1))
                        nc.vector.tensor_add(t_C[:, 128 * l:128 * (l + 1)],
                                             t_C[:, 128 * l:128 * (l + 1)], cg)
                    s0 = pss.tile([128, 1], F32)
                    nc.tensor.matmul(s0, x_c[:, 0:128], t_onescol[:],
                                     start=True, stop=True)
                    nc.vector.tensor_add(t_S0, t_S0, s0)

            # =========== Phase B: stats extraction, AllReduce dispatch ======
            with (
                tc.tile_pool(name="ps_fn", bufs=1, space="PSUM") as psfn,
                tc.tile_pool(name="ps_small", bufs=2, space="PSUM") as pssm,
            ):
                fn_ps = psfn.tile([128, 6], F32, tag="persist")
                for l in range(6):
                    t_ps = pssm.tile([128, 128], F32, tag="T")
                    nc.tensor.matmul(t_ps, t_C[:, 128 * l:128 * (l + 1)],
                                     t_w1[:, 128 * l:128 * (l + 1)],
                                     start=True, stop=True)
                    nc.vector.tensor_mul(t_P, t_w1[:, 128 * l:128 * (l + 1)], t_ps)
                    nc.tensor.matmul(fn_ps[:, l:l + 1], t_P[:],
                                     t_fnsc[:, l:l + 1], start=True, stop=True)
                ybar_ps = pssm.tile([128, 1], F32, tag="T")
                nc.tensor.matmul(ybar_ps, t_w1[:, 0:128], t_S0[:],
                                 start=True, stop=True)
                nc.vector.tensor_copy(t_stats[:, 0:6], fn_ps)
                nc.scalar.mul(out=t_stats[:, 6:7], in_=ybar_ps, mul=INV / B)

                with tc.tile_pool(name="dram", bufs=1, space="DRAM") as dpool:
                    cc_in = dpool.tile([128, 8], F32)
                    cc_out = dpool.tile([128, 8], F32)
                    nc.gpsimd.dma_start(out=cc_in[:], in_=t_stats[:])
                    nc.gpsimd.collective_compute(
                        "AllReduce",
                        mybir.AluOpType.add,
                        replica_groups=[list(range(NCORES))],
                        ins=[cc_in[:].opt()],
                        outs=[cc_out[:].opt()],
                    )
                    nc.gpsimd.dma_start(out=t_statsg[:], in_=cc_out[:])

                # ===== Transposes (second read of x) — overlap the AllReduce
                with tc.tile_pool(name="ps_tr", bufs=4, space="PSUM") as pst:
                    for c in range(BCHUNKS):
                        x2 = xin.tile([128, FEAT], CD, tag="x")
                        nc.sync.dma_start(out=x2[:], in_=xp[128 * c:128 * (c + 1), :])
                        xt_c = xtp.tile([128, FEAT], CD, tag="xt")
                        for p in range(NPAIR):
                            tp = pst.tile([128, 128], F32)
                            nc.tensor.matmul(tp, x2[:, 128 * p:128 * (p + 1)],
                                             t_ident[:], start=True, stop=True)
                            dst = xt_c[:, 128 * p:128 * (p + 1)]
                            if p % 2 == 0:
                                nc.vector.tensor_copy(dst, tp)
                            else:
                                nc.scalar.copy(out=dst, in_=tp)
                        x_tiles.append(xt_c)
                xts = x_tiles[BCHUNKS:]

                # ---- post-reduce: s, q, A, c0, Psi ----
                # fn0 -= ybar^2 ; s = bn_w / sqrt(fn + eps) ; q = s * W2^T * inv^2
                nc.vector.tensor_mul(t_tmp2, t_statsg[:, 6:7], t_statsg[:, 6:7])
                nc.vector.tensor_sub(t_statsg[:, 0:1], t_statsg[:, 0:1], t_tmp2)
                nc.scalar.activation(out=t_tmp1, in_=t_statsg[:, 0:6],
                                     func=mybir.ActivationFunctionType.Sqrt,
                                     bias=t_eps, scale=1.0)
                nc.vector.reciprocal(t_tmp1, t_tmp1)
                nc.vector.tensor_mul(t_tmp1, t_tmp1, t_bnwt)   # s [v, l]
                nc.vector.tensor_mul(t_tmp1, t_tmp1, t_w2s)    # q [v, l]

                a_ps = pssm.tile([128, 6], F32, tag="T")
                for l in range(6):
                    nc.tensor.matmul(a_ps[:, l:l + 1],
                                     t_w1t[:, 128 * l:128 * (l + 1)],
                                     t_tmp1[:, l:l + 1], start=True, stop=True)
                nc.vector.tensor_copy(t_A, a_ps)

                # c0 = sum_v (bnbw - ybar * q0 * sqrt(128))
                nc.vector.tensor_mul(t_tmp2, t_statsg[:, 6:7], t_tmp1[:, 0:1])
                nc.scalar.mul(out=t_tmp2, in_=t_tmp2, mul=math.sqrt(float(MUL)))
                nc.vector.tensor_sub(t_tmp2, t_bnbw, t_tmp2)
                t_tmp2c = work.tile([128, 1], CD)
                nc.vector.tensor_copy(t_tmp2c, t_tmp2)
                c0_ps = pssm.tile([1, 1], F32, tag="T")
                nc.tensor.matmul(c0_ps, t_tmp2c[:], t_onescol[:],
                                 start=True, stop=True)
                nc.vector.tensor_copy(t_c0row[0:1, 0:1], c0_ps)

                # Psi: column i of pair-p tile gets A[:, l(p)] (i == p)
                for p in range(NPAIR):
                    dst = t_psi[:, 36 * p + p:36 * p + p + 1]
                    src = t_A[:, L_OF[p]:L_OF[p] + 1]
                    if p % 2 == 0:
                        nc.vector.tensor_copy(dst, src)
                    else:
                        nc.scalar.copy(out=dst, in_=src)

            # =========== Phase C: z^T ==========
            with tc.tile_pool(name="ps_z", bufs=2, space="PSUM") as psz:
                for c in range(BCHUNKS):
                    zt_ps = psz.tile([36, 128], F32)
                    for p in range(NPAIR):
                        nc.tensor.matmul(zt_ps, t_psi[:, 36 * p:36 * (p + 1)],
                                         xts[c][:, 128 * p:128 * (p + 1)],
                                         start=(p == 0), stop=False)
                    nc.tensor.matmul(zt_ps, t_c0row[:], t_onesrow[:],
                                     start=False, stop=True)
                    nc.vector.tensor_copy(t_zts[0:36, 128 * c:128 * (c + 1)], zt_ps)
                # replicate z^T to partitions 64:100 for the odd-half chunks
                nc.sync.dma_start(out=t_zts[64:100, :], in_=t_zts[0:36, :])

            # =========== Phase D: S2 grid -> sigmoid -> z2 ==========
            with (
                tc.tile_pool(name="ps_grid", bufs=2, space="PSUM") as psgr,
                tc.tile_pool(name="ps_z2", bufs=1, space="PSUM") as psz2,
                tc.tile_pool(name="sg", bufs=2) as sgp,
            ):
                z2_ps = psz2.tile([36, BS], F32)
                groups = [list(range(s, min(s + SIG_GROUP, NGC)))
                          for s in range(0, NGC, SIG_GROUP)]
                for grp in groups:
                    nj = len(grp)
                    gr_ps = psgr.tile([128, SIG_GROUP, BS], F32, tag="grid")
                    sg = sgp.tile([128, SIG_GROUP, BS], CD, tag="sg")
                    for j, gc in enumerate(grp):
                        h, cblk = gc % 2, gc // 2
                        nc.tensor.matmul(
                            gr_ps[:, j, :],
                            t_yto2[64 * h:64 * h + 36, 128 * cblk:128 * (cblk + 1)],
                            t_zts[64 * h:64 * h + 36, :], start=True, stop=True)
                    nc.scalar.activation(out=sg[:, 0:nj, :], in_=gr_ps[:, 0:nj, :],
                                         func=mybir.ActivationFunctionType.Sigmoid)
                    for j, gc in enumerate(grp):
                        nc.tensor.matmul(z2_ps, t_yfromt[:, 36 * gc:36 * (gc + 1)],
                                         sg[:, j, :],
                                         start=(gc == 0), stop=(gc == NGC - 1))
                nc.vector.tensor_copy(t_z2s[0:36, :], z2_ps)
                nc.sync.dma_start(out=t_z2s[64:100, :], in_=t_z2s[0:36, :])

            # =========== Phase E: W3 expansion + output ==========
            with (
                tc.tile_pool(name="ps_out", bufs=3, space="PSUM") as pso,
                tc.tile_pool(name="osb", bufs=3) as osbp,
            ):
                for c in range(BCHUNKS):
                    for half, fcs in enumerate(([0, 1, 2, 3, 4], [5, 6, 7, 8])):
                        osb = osbp.tile([128, 5 * 512], F32, tag="osb")
                        for k, fc in enumerate(fcs):
                            h, cblk = fc % 2, fc // 2
                            o_ps = pso.tile([128, 512], F32)
                            nc.tensor.matmul(
                                o_ps,
                                t_z2s[64 * h:64 * h + 36, 128 * c:128 * (c + 1)],
                                t_e2[64 * h:64 * h + 36, 512 * cblk:512 * (cblk + 1)],
                                start=True, stop=True)
                            dst = osb[:, 512 * k:512 * (k + 1)]
                            if fc % 2 == 0:
                                nc.vector.tensor_copy(dst, o_ps)
                            else:
                                nc.scalar.copy(out=dst, in_=o_ps)
                        lo = 128 * c
                        c0_ = 2560 * half
                        n_ = 512 * len(fcs)
                        nc.sync.dma_start(out=out[lo:lo + 128, c0_:c0_ + n_],
                                          in_=osb[:, 0:n_])

    nc.compile()
    return nc


# ---------------------------------------------------------------------------
# Public entry point
# ---------------------------------------------------------------------------

_CACHE = {}


def _get_nc():
    if "nc" not in _CACHE:
        _CACHE["nc"] = build_nc()
    return _CACHE["nc"]


def make_in_maps(x, W1, W2, W3, bn_w, bn_b):
    consts = _host_constants()
    weights = _host_weights(W1, W2, W3, bn_w, bn_b)
    xp = _permute_x(x)
    shared = {**consts, **weights}
    in_maps = []
    for c in range(NCORES):
        m = dict(shared)
        m["xp"] = np.ascontiguousarray(xp[BS * c:BS * (c + 1)])
        in_maps.append(m)
    return in_maps


def kernel(x, W1, W2, W3, bn_w, bn_b, trace=False, **run_kwargs):
    nc = _get_nc()
    in_maps = make_in_maps(x, W1, W2, W3, bn_w, bn_b)
    res = run_bass_kernel_spmd(nc, in_maps, list(range(NCORES)),
                               trace=trace, **run_kwargs)
    out = np.concatenate([res.results[c]["out"] for c in range(NCORES)], axis=0)
    kernel.last_results = res
    return out.astype(np.float32)


# revision 19
# speedup vs baseline: 1.3153x; 1.1761x over previous
"""Trainium2 Bass kernel for nn_E3nnMLPBlockS2Grid.

Data-parallel over batch (B=4096 -> 512 rows/core on 8 cores).

Math restructuring (validated to ~2e-6 abs err against the jax reference):
  - e3nn BatchNorm statistics are computed WITHOUT materializing y = x @ W1:
    per-l Gram matrices C_l = sum_{b,m} x_l[:,u,m] x_l[:,u',m]^T are built on
    the TensorEngine contracting the batch (partition) dim, so x stays in its
    natural [b, feat] layout.  fn[v] = diag(W1^T C W1) * inv^2 / (B d), which is
    linear in C, so per-core partials are AllReduce'd (a [128,8] f32 tile).
  - After the reduce, the whole Linear->BN->Linear front-end collapses to one
    vector per l:  A[l][u] = inv^2 * sum_v W1[l][u,v] * s[l][v] * W2[l][v]
    plus a scalar c0 for the l=0 centered/biased path.
  - z^T[36, b] is accumulated in PSUM via per-(l,m) matmuls with sparse
    one-column Psi matrices (lhsT = Psi_lm [128u, 36], rhs = x^T_lm [128u, b]).
    The x^T tiles are built by PE transpose-matmuls against identity, scheduled
    inside the AllReduce window (x is read a second time from HBM for this so
    SBUF holds only a 2-chunk window of natural-layout x).
  - S2Activation: grid^T = YTO^T z^T (79 g-chunks of 128), sigmoid on ScalarE
    directly out of PSUM, z2^T accumulated with lhsT = YFROM^T chunks; the
    final W3 expansion is a dense [36, 4608] matmul producing the output in
    natural [b, feat] layout.

x is column-permuted on the host so each (l,m) 128-column block is contiguous.
COMPUTE_DT selects the streaming dtype (bfloat16 halves PE col time + DMA
bytes; all matmul accumulation and the statistics path stay fp32).
"""

import math

import numpy as np
import ml_dtypes

import concourse.bass as bass
import concourse.tile as tile
from concourse import bacc, mybir
from concourse.bass_utils import run_bass_kernel_spmd

F32 = mybir.dt.float32
F32R = mybir.dt.float32r
# Streaming dtype for x / S2 constants. bfloat16 halves PE-column time and DMA
# bytes but costs ~2e-2 max scale-relative error end-to-end (measured); fp32
# keeps it at ~1e-5. Correctness margin wins: fp32.  The big N=512 matmuls are
# issued as float32r (same 4-byte storage, PE replication mode: 1 cycle/row at
# N>=256 vs 4 cycles/row for plain fp32) via AP bitcasts at the call sites.
COMPUTE_DT = mybir.dt.float32
NP_COMPUTE = np.float32


def _tf32(a):
    """Round fp32 host data to tf32 (float32r): zero the low 13 mantissa bits
    with round-to-nearest so the PE replication mode sees pre-rounded values."""
    u = np.ascontiguousarray(a, np.float32).view(np.uint32)
    u = (u + 0x1000) & np.uint32(0xFFFFE000)
    return u.view(np.float32).copy()

MUL = 128
LMAX = 5
G = 100
NCORES = 8
B = 4096
BS = B // NCORES            # 512 rows per core
FEAT = 4608
EPS = 1e-5
INV = 1.0 / math.sqrt(MUL)
DIMS = [2 * l + 1 for l in range(LMAX + 1)]
OFFS = np.cumsum([0] + [MUL * d for d in DIMS]).tolist()
HOFF = np.cumsum([0] + DIMS).tolist()
NPAIR = 36                  # total (l, m) pairs == hidden dim
L_OF = np.concatenate([np.full(d, l) for l, d in enumerate(DIMS)]).tolist()
GG = G * G                  # 10000
NGC = 79                    # ceil(10000 / 128) g-chunks
BCHUNKS = BS // 128         # 4
SIG_GROUP = 3               # grid chunks per sigmoid call (3 PSUM banks)


# ---------------------------------------------------------------------------
# Host-side constants (S2 grid matrices etc. — identical math to the reference)
# ---------------------------------------------------------------------------

def _assoc_legendre(lmax, x):
    P = {(0, 0): np.ones_like(x)}
    s = np.sqrt(np.clip(1.0 - x * x, 0.0, None))
    for m in range(1, lmax + 1):
        P[(m, m)] = -(2 * m - 1) * s * P[(m - 1, m - 1)]
    for m in range(lmax):
        P[(m + 1, m)] = (2 * m + 1) * x * P[(m, m)]
    for m in range(lmax + 1):
        for l in range(m + 2, lmax + 1):
            P[(l, m)] = ((2 * l - 1) * x * P[(l - 1, m)] - (l + m - 1) * P[(l - 2, m)]) / (l - m)
    return P


def _build_s2_matrices():
    beta = (np.arange(G) + 0.5) * np.pi / G
    alpha = np.arange(G) * 2.0 * np.pi / G
    cb = np.cos(beta)
    P = _assoc_legendre(LMAX, cb)
    dim = (LMAX + 1) ** 2
    Y = np.zeros((dim, G, G))
    i = 0
    for l in range(LMAX + 1):
        for m in range(-l, l + 1):
            am = abs(m)
            N = math.sqrt((2 * l + 1) / (4 * math.pi) * math.factorial(l - am) / math.factorial(l + am))
            if m == 0:
                ang, c = np.ones(G), 1.0
            elif m > 0:
                ang, c = np.cos(m * alpha), math.sqrt(2.0)
            else:
                ang, c = np.sin(am * alpha), math.sqrt(2.0)
            Y[i] = c * N * P[(l, am)][:, None] * ang[None, :]
            i += 1
    V = np.polynomial.legendre.legvander(cb, G - 1).T
    e = np.zeros(G)
    e[0] = 2.0
    qw = np.linalg.solve(V, e)
    n_to = np.array([math.sqrt(4 * math.pi) * math.sqrt(2 * l + 1) / math.sqrt(LMAX + 1) for l in range(LMAX + 1)])
    lidx = np.concatenate([np.full(2 * l + 1, l, dtype=np.int64) for l in range(LMAX + 1)])
    Yto = (Y * n_to[lidx][:, None, None]).astype(np.float32)
    Yfrom = (Y * (1.0 / n_to)[lidx][:, None, None] * qw[None, :, None] * (2.0 * np.pi / G)).astype(np.float32)
    return Yto.reshape(dim, GG), Yfrom.reshape(dim, GG)


def _host_constants():
    """Data-independent constant arrays shipped as extra kernel inputs."""
    yto, yfrom = _build_s2_matrices()

    # YTO packed for lhsT use: chunk gc -> rows 64*(gc%2)..+36, cols 128*(gc//2)..+128
    yto_pad = np.zeros((36, NGC * 128), np.float32)
    yto_pad[:, :GG] = yto
    yto2 = np.zeros((100, 40 * 128), np.float32)
    for gc in range(NGC):
        h, cblk = gc % 2, gc // 2
        yto2[64 * h:64 * h + 36, 128 * cblk:128 * (cblk + 1)] = yto_pad[:, 128 * gc:128 * (gc + 1)]

    # YFROM^T packed: chunk gc -> [128(g within chunk), 36] at cols 36*gc (zero-padded g)
    yfromt = np.zeros((128, NGC * 36), np.float32)
    for gc in range(NGC):
        lo, hi = 128 * gc, min(128 * (gc + 1), GG)
        yfromt[: hi - lo, 36 * gc:36 * (gc + 1)] = yfrom[:, lo:hi].T

    ident = np.eye(128, dtype=np.float32)
    onescol = np.ones((128, 1), np.float32)
    onesrow = np.ones((1, BS), np.float32)
    # per-l scaling for the fn column-sum matmul: inv^2 / (B * d)
    fnsc = np.zeros((128, 6), np.float32)
    for l in range(6):
        fnsc[:, l] = INV * INV / (B * DIMS[l])
    return {
        "yto2": _tf32(yto2),
        "yfromt": _tf32(yfromt),
        "ident": ident,
        "onescol": onescol,
        "onesrow": onesrow,
        "fnsc": fnsc,
    }


def _host_weights(W1, W2, W3, bn_w, bn_b):
    """Weight-derived arrays (runtime inputs, transformed on host)."""
    W1 = np.asarray(W1, np.float32)
    W2 = np.asarray(W2, np.float32)
    W3 = np.asarray(W3, np.float32)
    bn_w = np.asarray(bn_w, np.float32)
    bn_b = np.asarray(bn_b, np.float32)

    w1 = np.zeros((128, 768), np.float32)      # [u, (l v)]
    w1t = np.zeros((128, 768), np.float32)     # [v, (l u)]
    for l in range(6):
        w1[:, 128 * l:128 * (l + 1)] = W1[l]
        w1t[:, 128 * l:128 * (l + 1)] = W1[l].T
    bnwt = bn_w.T.copy()                       # [128(v), 6]
    w2s = (W2.T * (INV * INV)).astype(np.float32)  # [128(v), 6]
    bnbw = (bn_b * W2[0] * INV).reshape(128, 1).astype(np.float32)

    # W3 replicated across partitions for the final broadcast-multiply:
    # out[b, OFFS_l + v*d + m] = z2[b, HOFF_l+m] * W3[l][v]
    w3bc = np.tile(np.concatenate([W3[l] for l in range(6)]).reshape(1, 768),
                   (128, 1)).astype(np.float32)
    return {
        "w1": w1,
        "w1t": w1t,
        "bnwt": bnwt,
        "w2s": w2s,
        "bnbw": bnbw,
        "w3bc": w3bc,
    }


def _permute_x(x):
    """Column-permute x so each (l, m) block of 128 u's is contiguous:
    new col index for pair p=(l,m): 128*p + u (old: OFFS[l] + u*d + m)."""
    perm = np.zeros(FEAT, np.int64)
    p = 0
    for l in range(6):
        d = DIMS[l]
        for m in range(d):
            perm[128 * p:128 * (p + 1)] = OFFS[l] + m + d * np.arange(128)
            p += 1
    return np.ascontiguousarray(np.asarray(x, np.float32)[:, perm].astype(NP_COMPUTE))


# ---------------------------------------------------------------------------
# The Bass program (SPMD, one NeuronCore shown; run on 8)
# ---------------------------------------------------------------------------

def build_nc():
    # Bacc (not raw Bass): its compile() pipeline splits multi-semaphore waits
    # (TRN2 matmuls support a single sync wait) via generate_event_semaphores.
    nc = bacc.Bacc(None, num_devices=NCORES)
    CD = COMPUTE_DT

    xp = nc.dram_tensor("xp", [BS, FEAT], CD, kind="ExternalInput")
    d_w1 = nc.dram_tensor("w1", [128, 768], F32, kind="ExternalInput")
    d_w1t = nc.dram_tensor("w1t", [128, 768], F32, kind="ExternalInput")
    d_bnwt = nc.dram_tensor("bnwt", [128, 6], F32, kind="ExternalInput")
    d_w2s = nc.dram_tensor("w2s", [128, 6], F32, kind="ExternalInput")
    d_bnbw = nc.dram_tensor("bnbw", [128, 1], F32, kind="ExternalInput")
    d_w3bc = nc.dram_tensor("w3bc", [128, 768], F32, kind="ExternalInput")
    d_yto2 = nc.dram_tensor("yto2", [100, 5120], F32R, kind="ExternalInput")
    d_yfromt = nc.dram_tensor("yfromt", [128, NGC * 36], F32R, kind="ExternalInput")
    d_ident = nc.dram_tensor("ident", [128, 128], CD, kind="ExternalInput")
    d_onescol = nc.dram_tensor("onescol", [128, 1], CD, kind="ExternalInput")
    d_onesrow = nc.dram_tensor("onesrow", [1, BS], CD, kind="ExternalInput")
    d_fnsc = nc.dram_tensor("fnsc", [128, 6], F32, kind="ExternalInput")

    out = nc.dram_tensor("out", [BS, FEAT], F32, kind="ExternalOutput")

    with tile.TileContext(nc) as tc:
        with (
            tc.tile_pool(name="consts", bufs=1) as consts,
            tc.tile_pool(name="work", bufs=1) as work,
            tc.tile_pool(name="xin", bufs=2) as xin,
            tc.tile_pool(name="xt", bufs=1) as xtp,
        ):
            # ---- x chunk 0 + phase-A constants first (critical path) ----
            t_ident = consts.tile([128, 128], CD)
            t_onescol = consts.tile([128, 1], CD)
            x_tiles = []
            x0 = xin.tile([128, FEAT], CD, tag="x")
            nc.sync.dma_start(out=x0[:], in_=xp[0:128, :])
            nc.scalar.dma_start(out=t_onescol[:], in_=d_onescol[:])
            x_tiles.append(x0)
            for c in range(1, BCHUNKS):
                x_c = xin.tile([128, FEAT], CD, tag="x")
                nc.sync.dma_start(out=x_c[:], in_=xp[128 * c:128 * (c + 1), :])
                x_tiles.append(x_c)

            # ---- remaining constants (scalar-engine HWDGE queue) ----
            t_w1 = consts.tile([128, 768], F32)
            t_w1t = consts.tile([128, 768], F32)
            t_bnwt = consts.tile([128, 6], F32)
            t_w2s = consts.tile([128, 6], F32)
            t_bnbw = consts.tile([128, 1], F32)
            t_fnsc = consts.tile([128, 6], F32)
            t_yto2 = consts.tile([100, 5120], F32R)
            t_yfromt = consts.tile([128, NGC * 36], F32R)
            t_w3bc = consts.tile([128, 768], F32)
            t_onesrow = consts.tile([1, BS], CD)
            for t, d in [
                (t_w1, d_w1), (t_fnsc, d_fnsc), (t_ident, d_ident),
                (t_onesrow, d_onesrow),
                (t_w1t, d_w1t), (t_bnwt, d_bnwt), (t_w2s, d_w2s),
                (t_bnbw, d_bnbw), (t_yto2, d_yto2), (t_yfromt, d_yfromt),
                (t_w3bc, d_w3bc),
            ]:
                nc.scalar.dma_start(out=t[:], in_=d[:])

            # ---- persistent SBUF work tiles ----
            t_C = work.tile([128, 768], F32)      # Gram accumulators [u, (l u')]
            t_S0 = work.tile([128, 1], F32)       # sum_b x_0[b, u]
            t_stats = work.tile([128, 8], F32)    # AR payload
            t_statsg = work.tile([128, 8], F32)   # AR result
            t_eps = work.tile([128, 1], F32)
            t_P = work.tile([128, 128], F32)
            t_A = work.tile([128, 6], F32)
            t_psi = work.tile([128, NPAIR * 36], F32R)
            t_c0row = work.tile([1, 36], CD)
            # z^T lives at partitions 0:36 and is replicated to 64:100
            # (matmul requires lhsT/rhs to share a base partition, and the
            # yto2 constant is packed two chunks per 128 partitions).
            t_zts = work.tile([100, BS], F32R)    # z^T
            t_z2s = work.tile([36, BS], F32)      # z2^T
            t_z2n = work.tile([128, BCHUNKS, 36], F32)  # z2 transposed [b, i]
            t_tmp1 = work.tile([128, 6], F32)
            t_tmp2 = work.tile([128, 1], F32)

            nc.vector.memset(t_C, 0.0)
            nc.vector.memset(t_S0, 0.0)
            nc.vector.memset(t_stats, 0.0)
            nc.vector.memset(t_eps, EPS)
            nc.gpsimd.memset(t_psi[:].bitcast(F32), 0.0)
            nc.gpsimd.memset(t_c0row, 0.0)

            # =========== Phase A: Gram + S0 partials ==========
            with (
                tc.tile_pool(name="ps_gram", bufs=2, space="PSUM") as psg,
                tc.tile_pool(name="ps_s0", bufs=2, space="PSUM") as pss,
            ):
                for c in range(BCHUNKS):
                    x_c = x_tiles[c]
                    for l in range(6):
                        d = DIMS[l]
                        cg = psg.tile([128, 128], F32)
                        for m in range(d):
                            p = HOFF[l] + m
                            sl = x_c[:, 128 * p:128 * (p + 1)]
                            nc.tensor.matmul(cg, sl, sl,
                                             start=(m == 0), stop=(m == d - 1))
                        nc.vector.tensor_add(t_C[:, 128 * l:128 * (l + 1)],
                                             t_C[:, 128 * l:128 * (l + 1)], cg)
                    s0 = pss.tile([128, 1], F32)
                    nc.tensor.matmul(s0, x_c[:, 0:128], t_onescol[:],
                                     start=True, stop=True)
                    nc.vector.tensor_add(t_S0, t_S0, s0)

            # =========== Phase B: stats extraction, AllReduce dispatch ======
            with (
                tc.tile_pool(name="ps_fn", bufs=1, space="PSUM") as psfn,
                tc.tile_pool(name="ps_small", bufs=2, space="PSUM") as pssm,
            ):
                fn_ps = psfn.tile([128, 6], F32, tag="persist")
                for l in range(6):
                    t_ps = pssm.tile([128, 128], F32, tag="T")
                    nc.tensor.matmul(t_ps, t_C[:, 128 * l:128 * (l + 1)],
                                     t_w1[:, 128 * l:128 * (l + 1)],
                                     start=True, stop=True)
                    nc.vector.tensor_mul(t_P, t_w1[:, 128 * l:128 * (l + 1)], t_ps)
                    nc.tensor.matmul(fn_ps[:, l:l + 1], t_P[:],
                                     t_fnsc[:, l:l + 1], start=True, stop=True)
                ybar_ps = pssm.tile([128, 1], F32, tag="T")
                nc.tensor.matmul(ybar_ps, t_w1[:, 0:128], t_S0[:],
                                 start=True, stop=True)
                nc.vector.tensor_copy(t_stats[:, 0:6], fn_ps)
                nc.scalar.mul(out=t_stats[:, 6:7], in_=ybar_ps, mul=INV / B)

                with tc.tile_pool(name="dram", bufs=1, space="DRAM") as dpool:
                    cc_in = dpool.tile([128, 8], F32)
                    cc_out = dpool.tile([128, 8], F32)
                    nc.gpsimd.dma_start(out=cc_in[:], in_=t_stats[:])
                    nc.gpsimd.collective_compute(
                        "AllReduce",
                        mybir.AluOpType.add,
                        replica_groups=[list(range(NCORES))],
                        ins=[cc_in[:].opt()],
                        outs=[cc_out[:].opt()],
                    )
                    nc.gpsimd.dma_start(out=t_statsg[:], in_=cc_out[:])

                # ===== Transposes (second read of x) — overlap the AllReduce
                xt_all = xtp.tile([128, NPAIR, BS], F32R, tag="xt")
                with tc.tile_pool(name="ps_tr", bufs=4, space="PSUM") as pst:
                    for c in range(BCHUNKS):
                        x2 = xin.tile([128, FEAT], CD, tag="x")
                        nc.sync.dma_start(out=x2[:], in_=xp[128 * c:128 * (c + 1), :])
                        for p in range(NPAIR):
                            tp = pst.tile([128, 128], F32)
                            nc.tensor.transpose(tp, x2[:, 128 * p:128 * (p + 1)],
                                                t_ident[:])
                            dst = xt_all[:, p, 128 * c:128 * (c + 1)]
                            if p % 2 == 0:
                                nc.vector.tensor_copy(dst, tp)
                            else:
                                nc.scalar.copy(out=dst, in_=tp)

                # ---- post-reduce: s, q, A, c0, Psi ----
                # fn0 -= ybar^2 ; s = bn_w / sqrt(fn + eps) ; q = s * W2^T * inv^2
                nc.vector.tensor_mul(t_tmp2, t_statsg[:, 6:7], t_statsg[:, 6:7])
                nc.vector.tensor_sub(t_statsg[:, 0:1], t_statsg[:, 0:1], t_tmp2)
                nc.scalar.activation(out=t_tmp1, in_=t_statsg[:, 0:6],
                                     func=mybir.ActivationFunctionType.Sqrt,
                                     bias=t_eps, scale=1.0)
                nc.vector.reciprocal(t_tmp1, t_tmp1)
                nc.vector.tensor_mul(t_tmp1, t_tmp1, t_bnwt)   # s [v, l]
                nc.vector.tensor_mul(t_tmp1, t_tmp1, t_w2s)    # q [v, l]

                a_ps = pssm.tile([128, 6], F32, tag="T")
                for l in range(6):
                    nc.tensor.matmul(a_ps[:, l:l + 1],
                                     t_w1t[:, 128 * l:128 * (l + 1)],
                                     t_tmp1[:, l:l + 1], start=True, stop=True)
                nc.vector.tensor_copy(t_A, a_ps)

                # c0 = sum_v (bnbw - ybar * q0 * sqrt(128))
                nc.vector.tensor_mul(t_tmp2, t_statsg[:, 6:7], t_tmp1[:, 0:1])
                nc.scalar.mul(out=t_tmp2, in_=t_tmp2, mul=math.sqrt(float(MUL)))
                nc.vector.tensor_sub(t_tmp2, t_bnbw, t_tmp2)
                t_tmp2c = work.tile([128, 1], CD)
                nc.vector.tensor_copy(t_tmp2c, t_tmp2)
                c0_ps = pssm.tile([1, 1], F32, tag="T")
                nc.tensor.matmul(c0_ps, t_tmp2c[:], t_onescol[:],
                                 start=True, stop=True)
                nc.vector.tensor_copy(t_c0row[0:1, 0:1], c0_ps)

                # Psi: column i of pair-p tile gets A[:, l(p)] (i == p)
                for p in range(NPAIR):
                    dst = t_psi[:, 36 * p + p:36 * p + p + 1]
                    src = t_A[:, L_OF[p]:L_OF[p] + 1]
                    if p % 2 == 0:
                        nc.vector.tensor_copy(dst, src)
                    else:
                        nc.scalar.copy(out=dst, in_=src)

            # =========== Phase C: z^T ==========
            with tc.tile_pool(name="ps_z", bufs=1, space="PSUM") as psz:
                zt_ps = psz.tile([36, BS], F32)
                for p in range(NPAIR):
                    nc.tensor.matmul(zt_ps, t_psi[:, 36 * p:36 * (p + 1)],
                                     xt_all[:, p, :],
                                     start=(p == 0), stop=False)
                nc.tensor.matmul(zt_ps, t_c0row[:], t_onesrow[:],
                                 start=False, stop=True)
                nc.vector.tensor_copy(t_zts[0:36, :], zt_ps)
                # replicate z^T to partitions 64:100 for the odd-half chunks
                nc.sync.dma_start(out=t_zts[64:100, :], in_=t_zts[0:36, :])

            # =========== Phase D: S2 grid -> sigmoid -> z2 ==========
            with (
                tc.tile_pool(name="ps_grid", bufs=2, space="PSUM") as psgr,
                tc.tile_pool(name="ps_z2", bufs=1, space="PSUM") as psz2,
                tc.tile_pool(name="sg", bufs=2) as sgp,
            ):
                z2_ps = psz2.tile([36, BS], F32)
                groups = [list(range(s, min(s + SIG_GROUP, NGC)))
                          for s in range(0, NGC, SIG_GROUP)]
                for grp in groups:
                    nj = len(grp)
                    gr_ps = psgr.tile([128, SIG_GROUP, BS], F32, tag="grid")
                    sg = sgp.tile([128, SIG_GROUP, BS], F32R, tag="sg")
                    for j, gc in enumerate(grp):
                        h, cblk = gc % 2, gc // 2
                        nc.tensor.matmul(
                            gr_ps[:, j, :],
                            t_yto2[64 * h:64 * h + 36, 128 * cblk:128 * (cblk + 1)],
                            t_zts[64 * h:64 * h + 36, :], start=True, stop=True)
                    nc.scalar.activation(out=sg[:, 0:nj, :], in_=gr_ps[:, 0:nj, :],
                                         func=mybir.ActivationFunctionType.Sigmoid)
                    for j, gc in enumerate(grp):
                        nc.tensor.matmul(z2_ps, t_yfromt[:, 36 * gc:36 * (gc + 1)],
                                         sg[:, j, :],
                                         start=(gc == 0), stop=(gc == NGC - 1))
                nc.vector.tensor_copy(t_z2s[:], z2_ps)

            # =========== Phase E: W3 expansion + output ==========
            # out[b, OFFS_l + v*d + m] = z2[b, HOFF_l + m] * W3[l][v] — a
            # broadcast multiply, done in exact fp32 as tensor_scalar ops
            # (in0 = replicated W3 row block, scalar = z2 column) spread
            # across Vector/Scalar/GpSimd.  Needs z2 as [b, i]: 4 PE
            # transposes of the [36, 128] z2^T chunks.
            with (
                tc.tile_pool(name="ps_zn", bufs=2, space="PSUM") as psn,
                tc.tile_pool(name="osb", bufs=2) as osbp,
            ):
                for c in range(BCHUNKS):
                    zn_ps = psn.tile([128, 36], F32)
                    nc.tensor.transpose(zn_ps, t_z2s[:, 128 * c:128 * (c + 1)],
                                        t_ident[0:36, 0:36])
                    nc.vector.tensor_copy(t_z2n[:, c, :], zn_ps)
                for c in range(BCHUNKS):
                    osb = osbp.tile([128, FEAT], F32, tag="osb")
                    for p in range(NPAIR):
                        l = L_OF[p]
                        d = DIMS[l]
                        m = p - HOFF[l]
                        blk = osb[:, OFFS[l]:OFFS[l + 1]].rearrange(
                            "b (v m) -> b m v", m=d)[:, m, :]
                        w3b = t_w3bc[:, 128 * l:128 * (l + 1)]
                        sc = t_z2n[:, c, p:p + 1]
                        eng = (nc.vector, nc.scalar, nc.gpsimd)[p % 3]
                        if eng is nc.scalar:
                            nc.scalar.activation(
                                out=blk, in_=w3b,
                                func=mybir.ActivationFunctionType.Copy,
                                scale=sc)
                        else:
                            eng.tensor_scalar_mul(blk, w3b, sc)
                    lo = 128 * c
                    nc.sync.dma_start(out=out[lo:lo + 128, :], in_=osb[:])

    nc.compile()
    return nc


# ---------------------------------------------------------------------------
# Public entry point
# ---------------------------------------------------------------------------

_CACHE = {}


def _get_nc():
    if "nc" not in _CACHE:
        _CACHE["nc"] = build_nc()
    return _CACHE["nc"]


def make_in_maps(x, W1, W2, W3, bn_w, bn_b):
    consts = _host_constants()
    weights = _host_weights(W1, W2, W3, bn_w, bn_b)
    xp = _permute_x(x)
    shared = {**consts, **weights}
    in_maps = []
    for c in range(NCORES):
        m = dict(shared)
        m["xp"] = np.ascontiguousarray(xp[BS * c:BS * (c + 1)])
        in_maps.append(m)
    return in_maps


def kernel(x, W1, W2, W3, bn_w, bn_b, trace=False, **run_kwargs):
    nc = _get_nc()
    in_maps = make_in_maps(x, W1, W2, W3, bn_w, bn_b)
    res = run_bass_kernel_spmd(nc, in_maps, list(range(NCORES)),
                               trace=trace, **run_kwargs)
    out = np.concatenate([res.results[c]["out"] for c in range(NCORES)], axis=0)
    kernel.last_results = res
    return out.astype(np.float32)


# revision 21
# speedup vs baseline: 1.4891x; 1.1321x over previous
"""Trainium2 Bass kernel for nn_E3nnMLPBlockS2Grid.

Data-parallel over batch (B=4096 -> 512 rows/core on 8 cores).

Math restructuring (validated to ~2e-6 abs err against the jax reference):
  - e3nn BatchNorm statistics are computed WITHOUT materializing y = x @ W1:
    per-l Gram matrices C_l = sum_{b,m} x_l[:,u,m] x_l[:,u',m]^T are built on
    the TensorEngine contracting the batch (partition) dim, so x stays in its
    natural [b, feat] layout.  fn[v] = diag(W1^T C W1) * inv^2 / (B d), which is
    linear in C, so per-core partials are AllReduce'd (a [128,8] f32 tile).
  - After the reduce, the whole Linear->BN->Linear front-end collapses to one
    vector per l:  A[l][u] = inv^2 * sum_v W1[l][u,v] * s[l][v] * W2[l][v]
    plus a scalar c0 for the l=0 centered/biased path.
  - z^T[36, b] is accumulated in PSUM via per-(l,m) matmuls with sparse
    one-column Psi matrices (lhsT = Psi_lm [128u, 36], rhs = x^T_lm [128u, b]).
    The x^T tiles are built by PE transpose-matmuls against identity, scheduled
    inside the AllReduce window (x is read a second time from HBM for this so
    SBUF holds only a 2-chunk window of natural-layout x).
  - S2Activation: grid^T = YTO^T z^T (79 g-chunks of 128), sigmoid on ScalarE
    directly out of PSUM, z2^T accumulated with lhsT = YFROM^T chunks; the
    final W3 expansion is a dense [36, 4608] matmul producing the output in
    natural [b, feat] layout.

x is column-permuted on the host so each (l,m) 128-column block is contiguous.
COMPUTE_DT selects the streaming dtype (bfloat16 halves PE col time + DMA
bytes; all matmul accumulation and the statistics path stay fp32).
"""

import math

import numpy as np
import ml_dtypes

import concourse.bass as bass
import concourse.tile as tile
from concourse import bacc, mybir
from concourse.bass_utils import run_bass_kernel_spmd

F32 = mybir.dt.float32
F32R = mybir.dt.float32r
# Streaming dtype for x / S2 constants. bfloat16 halves PE-column time and DMA
# bytes but costs ~2e-2 max scale-relative error end-to-end (measured); fp32
# keeps it at ~1e-5. Correctness margin wins: fp32.  The big N=512 matmuls are
# issued as float32r (same 4-byte storage, PE replication mode: 1 cycle/row at
# N>=256 vs 4 cycles/row for plain fp32) via AP bitcasts at the call sites.
COMPUTE_DT = mybir.dt.float32
NP_COMPUTE = np.float32


def _tf32(a):
    """Round fp32 host data to tf32 (float32r): zero the low 13 mantissa bits
    with round-to-nearest so the PE replication mode sees pre-rounded values."""
    u = np.ascontiguousarray(a, np.float32).view(np.uint32)
    u = (u + 0x1000) & np.uint32(0xFFFFE000)
    return u.view(np.float32).copy()

MUL = 128
LMAX = 5
G = 100
NCORES = 8
B = 4096
BS = B // NCORES            # 512 rows per core
FEAT = 4608
EPS = 1e-5
INV = 1.0 / math.sqrt(MUL)
DIMS = [2 * l + 1 for l in range(LMAX + 1)]
OFFS = np.cumsum([0] + [MUL * d for d in DIMS]).tolist()
HOFF = np.cumsum([0] + DIMS).tolist()
NPAIR = 36                  # total (l, m) pairs == hidden dim
L_OF = np.concatenate([np.full(d, l) for l, d in enumerate(DIMS)]).tolist()
GG = G * G                  # 10000
NGC = 79                    # ceil(10000 / 128) g-chunks
BCHUNKS = BS // 128         # 4
SIG_GROUP = 3               # grid chunks per sigmoid call (3 PSUM banks)


# ---------------------------------------------------------------------------
# Host-side constants (S2 grid matrices etc. — identical math to the reference)
# ---------------------------------------------------------------------------

def _assoc_legendre(lmax, x):
    P = {(0, 0): np.ones_like(x)}
    s = np.sqrt(np.clip(1.0 - x * x, 0.0, None))
    for m in range(1, lmax + 1):
        P[(m, m)] = -(2 * m - 1) * s * P[(m - 1, m - 1)]
    for m in range(lmax):
        P[(m + 1, m)] = (2 * m + 1) * x * P[(m, m)]
    for m in range(lmax + 1):
        for l in range(m + 2, lmax + 1):
            P[(l, m)] = ((2 * l - 1) * x * P[(l - 1, m)] - (l + m - 1) * P[(l - 2, m)]) / (l - m)
    return P


def _build_s2_matrices():
    beta = (np.arange(G) + 0.5) * np.pi / G
    alpha = np.arange(G) * 2.0 * np.pi / G
    cb = np.cos(beta)
    P = _assoc_legendre(LMAX, cb)
    dim = (LMAX + 1) ** 2
    Y = np.zeros((dim, G, G))
    i = 0
    for l in range(LMAX + 1):
        for m in range(-l, l + 1):
            am = abs(m)
            N = math.sqrt((2 * l + 1) / (4 * math.pi) * math.factorial(l - am) / math.factorial(l + am))
            if m == 0:
                ang, c = np.ones(G), 1.0
            elif m > 0:
                ang, c = np.cos(m * alpha), math.sqrt(2.0)
            else:
                ang, c = np.sin(am * alpha), math.sqrt(2.0)
            Y[i] = c * N * P[(l, am)][:, None] * ang[None, :]
            i += 1
    V = np.polynomial.legendre.legvander(cb, G - 1).T
    e = np.zeros(G)
    e[0] = 2.0
    qw = np.linalg.solve(V, e)
    n_to = np.array([math.sqrt(4 * math.pi) * math.sqrt(2 * l + 1) / math.sqrt(LMAX + 1) for l in range(LMAX + 1)])
    lidx = np.concatenate([np.full(2 * l + 1, l, dtype=np.int64) for l in range(LMAX + 1)])
    Yto = (Y * n_to[lidx][:, None, None]).astype(np.float32)
    Yfrom = (Y * (1.0 / n_to)[lidx][:, None, None] * qw[None, :, None] * (2.0 * np.pi / G)).astype(np.float32)
    return Yto.reshape(dim, GG), Yfrom.reshape(dim, GG)


def _host_constants():
    """Data-independent constant arrays shipped as extra kernel inputs."""
    yto, yfrom = _build_s2_matrices()

    # YTO packed for lhsT use: chunk gc -> rows 64*(gc%2)..+36, cols 128*(gc//2)..+128
    yto_pad = np.zeros((36, NGC * 128), np.float32)
    yto_pad[:, :GG] = yto
    yto2 = np.zeros((100, 40 * 128), np.float32)
    for gc in range(NGC):
        h, cblk = gc % 2, gc // 2
        yto2[64 * h:64 * h + 36, 128 * cblk:128 * (cblk + 1)] = yto_pad[:, 128 * gc:128 * (gc + 1)]

    # YFROM^T packed: chunk gc -> [128(g within chunk), 36] at cols 36*gc (zero-padded g)
    yfromt = np.zeros((128, NGC * 36), np.float32)
    for gc in range(NGC):
        lo, hi = 128 * gc, min(128 * (gc + 1), GG)
        yfromt[: hi - lo, 36 * gc:36 * (gc + 1)] = yfrom[:, lo:hi].T

    ident = np.eye(128, dtype=np.float32)
    onescol = np.zeros((128, 2), np.float32)
    onescol[:, 0] = 1.0
    one32 = np.ones((128, 1), np.float32)
    onesrow = np.ones((1, BS), np.float32)
    # per-l scaling for the fn column-sum matmul: inv^2 / (B * d)
    fnsc = np.zeros((128, 6), np.float32)
    for l in range(6):
        fnsc[:, l] = INV * INV / (B * DIMS[l])
    return {
        "yto2": _tf32(yto2),
        "yfromt": _tf32(yfromt),
        "ident": ident,
        "identpad": np.concatenate([ident, np.zeros((128, 128), np.float32)], axis=1),
        "onescol": onescol,
        "one32": one32,
        "onesrow": onesrow,
        "fnsc": fnsc,
    }


def _host_weights(W1, W2, W3, bn_w, bn_b):
    """Weight-derived arrays (runtime inputs, transformed on host)."""
    W1 = np.asarray(W1, np.float32)
    W2 = np.asarray(W2, np.float32)
    W3 = np.asarray(W3, np.float32)
    bn_w = np.asarray(bn_w, np.float32)
    bn_b = np.asarray(bn_b, np.float32)

    w1 = np.zeros((128, 768), np.float32)      # [u, (l v)]
    w1t = np.zeros((128, 768), np.float32)     # [v, (l u)]
    for l in range(6):
        w1[:, 128 * l:128 * (l + 1)] = W1[l]
        w1t[:, 128 * l:128 * (l + 1)] = W1[l].T
    bnwt = bn_w.T.copy()                       # [128(v), 6]
    w2s = (W2.T * (INV * INV)).astype(np.float32)  # [128(v), 6]
    bnbw = (bn_b * W2[0] * INV).reshape(128, 1).astype(np.float32)

    # W3 replicated across partitions for the final broadcast-multiply:
    # out[b, OFFS_l + v*d + m] = z2[b, HOFF_l+m] * W3[l][v]
    w3bc = np.tile(np.concatenate([W3[l] for l in range(6)]).reshape(1, 768),
                   (128, 1)).astype(np.float32)
    return {
        "w1": w1,
        "w1t": w1t,
        "bnwt": bnwt,
        "w2s": w2s,
        "bnbw": bnbw,
        "w3bc": w3bc,
    }


def _perm():
    """Pair-major column permutation: new col 128*p + u <-> old OFFS_l + u*d + m."""
    perm = np.zeros(FEAT, np.int64)
    p = 0
    for l in range(6):
        d = DIMS[l]
        for m in range(d):
            perm[128 * p:128 * (p + 1)] = OFFS[l] + m + d * np.arange(128)
            p += 1
    return perm


def _permute_x(x):
    """Permute x pair-major, round to tf32, and pad 128 zero cols (so the
    N=256 sliding-window Gram matmuls can read one block past the end)."""
    xp = np.zeros((B, FEAT + 128), np.float32)
    xp[:, :FEAT] = np.asarray(x, np.float32)[:, _perm()]
    return _tf32(xp)


# ---------------------------------------------------------------------------
# The Bass program (SPMD, one NeuronCore shown; run on 8)
# ---------------------------------------------------------------------------

def build_nc():
    # Bacc (not raw Bass): its compile() pipeline splits multi-semaphore waits
    # (TRN2 matmuls support a single sync wait) via generate_event_semaphores.
    nc = bacc.Bacc(None, num_devices=NCORES)
    CD = COMPUTE_DT

    xp = nc.dram_tensor("xp", [BS, FEAT + 128], F32R, kind="ExternalInput")
    d_w1 = nc.dram_tensor("w1", [128, 768], F32, kind="ExternalInput")
    d_w1t = nc.dram_tensor("w1t", [128, 768], F32, kind="ExternalInput")
    d_bnwt = nc.dram_tensor("bnwt", [128, 6], F32, kind="ExternalInput")
    d_w2s = nc.dram_tensor("w2s", [128, 6], F32, kind="ExternalInput")
    d_bnbw = nc.dram_tensor("bnbw", [128, 1], F32, kind="ExternalInput")
    d_w3bc = nc.dram_tensor("w3bc", [128, 768], F32, kind="ExternalInput")
    d_yto2 = nc.dram_tensor("yto2", [100, 5120], F32R, kind="ExternalInput")
    d_yfromt = nc.dram_tensor("yfromt", [128, NGC * 36], F32R, kind="ExternalInput")
    d_ident = nc.dram_tensor("ident", [128, 128], CD, kind="ExternalInput")
    d_identpad = nc.dram_tensor("identpad", [128, 256], F32R, kind="ExternalInput")
    d_onescol = nc.dram_tensor("onescol", [128, 2], F32R, kind="ExternalInput")
    d_one32 = nc.dram_tensor("one32", [128, 1], F32, kind="ExternalInput")
    d_onesrow = nc.dram_tensor("onesrow", [1, BS], CD, kind="ExternalInput")
    d_fnsc = nc.dram_tensor("fnsc", [128, 6], F32, kind="ExternalInput")

    out = nc.dram_tensor("out", [BS, FEAT], F32, kind="ExternalOutput")

    with tile.TileContext(nc) as tc:
        with (
            tc.tile_pool(name="consts", bufs=1) as consts,
            tc.tile_pool(name="work", bufs=1) as work,
            tc.tile_pool(name="xin", bufs=2) as xin,
            tc.tile_pool(name="xt", bufs=1) as xtp,
        ):
            # ---- x chunk 0 + phase-A constants first (critical path) ----
            t_ident = consts.tile([128, 128], CD)
            t_identpad = consts.tile([128, 256], F32R)
            t_onescol = consts.tile([128, 2], F32R)
            t_one32 = consts.tile([128, 1], F32)
            x_tiles = []
            x0 = xin.tile([128, FEAT + 128], F32R, tag="x")
            nc.sync.dma_start(out=x0[:], in_=xp[0:128, :])
            nc.scalar.dma_start(out=t_onescol[:], in_=d_onescol[:])
            nc.scalar.dma_start(out=t_one32[:], in_=d_one32[:])
            x_tiles.append(x0)
            for c in range(1, BCHUNKS):
                x_c = xin.tile([128, FEAT + 128], F32R, tag="x")
                nc.sync.dma_start(out=x_c[:], in_=xp[128 * c:128 * (c + 1), :])
                x_tiles.append(x_c)

            # ---- remaining constants (scalar-engine HWDGE queue) ----
            t_w1 = consts.tile([128, 768], F32)
            t_w1t = consts.tile([128, 768], F32)
            t_bnwt = consts.tile([128, 6], F32)
            t_w2s = consts.tile([128, 6], F32)
            t_bnbw = consts.tile([128, 1], F32)
            t_fnsc = consts.tile([128, 6], F32)
            t_yto2 = consts.tile([100, 5120], F32R)
            t_yfromt = consts.tile([128, NGC * 36], F32R)
            t_w3bc = consts.tile([128, 768], F32)
            t_onesrow = consts.tile([1, BS], CD)
            for t, d in [
                (t_w1, d_w1), (t_fnsc, d_fnsc), (t_ident, d_ident),
                (t_identpad, d_identpad), (t_onesrow, d_onesrow),
                (t_w1t, d_w1t), (t_bnwt, d_bnwt), (t_w2s, d_w2s),
                (t_bnbw, d_bnbw), (t_yto2, d_yto2), (t_yfromt, d_yfromt),
                (t_w3bc, d_w3bc),
            ]:
                nc.scalar.dma_start(out=t[:], in_=d[:])

            # ---- persistent SBUF work tiles ----
            t_C = work.tile([128, 768], F32)      # Gram accumulators [u, (l u')]
            t_S0 = work.tile([128, 1], F32)       # sum_b x_0[b, u]
            t_stats = work.tile([128, 8], F32)    # AR payload
            t_statsg = work.tile([128, 8], F32)   # AR result
            t_eps = work.tile([128, 1], F32)
            t_P = work.tile([128, 128], F32)
            t_A = work.tile([128, 6], F32)
            t_psi = work.tile([128, NPAIR * 36], F32R)
            t_c0row = work.tile([1, 36], CD)
            # z^T lives at partitions 0:36 and is replicated to 64:100
            # (matmul requires lhsT/rhs to share a base partition, and the
            # yto2 constant is packed two chunks per 128 partitions).
            t_zts = work.tile([100, BS], F32R)    # z^T
            t_z2s = work.tile([36, BS], F32)      # z2^T
            t_z2n = work.tile([128, BCHUNKS, 36], F32)  # z2 transposed [b, i]
            t_tmp1 = work.tile([128, 6], F32)
            t_tmp2 = work.tile([128, 1], F32)

            nc.vector.memset(t_C, 0.0)
            nc.vector.memset(t_S0, 0.0)
            nc.vector.memset(t_stats, 0.0)
            nc.vector.memset(t_eps, EPS)
            nc.gpsimd.memset(t_psi[:].bitcast(F32), 0.0)
            nc.gpsimd.memset(t_c0row, 0.0)

            # =========== Phase A: Gram + S0 partials ==========
            with (
                tc.tile_pool(name="ps_gram", bufs=2, space="PSUM") as psg,
                tc.tile_pool(name="ps_s0", bufs=2, space="PSUM") as pss,
            ):
                for c in range(BCHUNKS):
                    x_c = x_tiles[c]
                    for l in range(6):
                        d = DIMS[l]
                        # N=256 windows keep float32r at 1 cycle/row; cols
                        # 128:256 accumulate cross-pair junk that is ignored.
                        cg = psg.tile([128, 256], F32)
                        for m in range(d):
                            p = HOFF[l] + m
                            nc.tensor.matmul(cg, x_c[:, 128 * p:128 * (p + 1)],
                                             x_c[:, 128 * p:128 * (p + 2)],
                                             start=(m == 0), stop=(m == d - 1))
                        nc.vector.tensor_add(t_C[:, 128 * l:128 * (l + 1)],
                                             t_C[:, 128 * l:128 * (l + 1)],
                                             cg[:, 0:128])
                    s0 = pss.tile([128, 2], F32)
                    nc.tensor.matmul(s0, x_c[:, 0:128], t_onescol[:],
                                     start=True, stop=True)
                    nc.vector.tensor_add(t_S0, t_S0, s0[:, 0:1])

            # =========== Phase B: stats extraction, AllReduce dispatch ======
            with (
                tc.tile_pool(name="ps_fn", bufs=1, space="PSUM") as psfn,
                tc.tile_pool(name="ps_small", bufs=2, space="PSUM") as pssm,
            ):
                fn_ps = psfn.tile([128, 6], F32, tag="persist")
                for l in range(6):
                    t_ps = pssm.tile([128, 128], F32, tag="T")
                    nc.tensor.matmul(t_ps, t_C[:, 128 * l:128 * (l + 1)],
                                     t_w1[:, 128 * l:128 * (l + 1)],
                                     start=True, stop=True)
                    nc.vector.tensor_mul(t_P, t_w1[:, 128 * l:128 * (l + 1)], t_ps)
                    nc.tensor.matmul(fn_ps[:, l:l + 1], t_P[:],
                                     t_fnsc[:, l:l + 1], start=True, stop=True)
                ybar_ps = pssm.tile([128, 1], F32, tag="T")
                nc.tensor.matmul(ybar_ps, t_w1[:, 0:128], t_S0[:],
                                 start=True, stop=True)
                nc.vector.tensor_copy(t_stats[:, 0:6], fn_ps)
                nc.scalar.mul(out=t_stats[:, 6:7], in_=ybar_ps, mul=INV / B)

                with tc.tile_pool(name="dram", bufs=1, space="DRAM") as dpool:
                    cc_in = dpool.tile([128, 8], F32)
                    cc_out = dpool.tile([128, 8], F32)
                    nc.gpsimd.dma_start(out=cc_in[:], in_=t_stats[:])
                    nc.gpsimd.collective_compute(
                        "AllReduce",
                        mybir.AluOpType.add,
                        replica_groups=[list(range(NCORES))],
                        ins=[cc_in[:].opt()],
                        outs=[cc_out[:].opt()],
                    )
                    nc.gpsimd.dma_start(out=t_statsg[:], in_=cc_out[:])

                # ===== Transposes (second read of x) — overlap the AllReduce
                xt_all = xtp.tile([128, NPAIR, BS], F32R, tag="xt")
                with tc.tile_pool(name="ps_tr", bufs=4, space="PSUM") as pst:
                    for c in range(BCHUNKS):
                        x2 = xin.tile([128, FEAT + 128], F32R, tag="x")
                        nc.sync.dma_start(out=x2[:], in_=xp[128 * c:128 * (c + 1), :])
                        for p in range(NPAIR):
                            tp = pst.tile([128, 256], F32)
                            nc.tensor.matmul(tp, x2[:, 128 * p:128 * (p + 1)],
                                             t_identpad[:], start=True, stop=True)
                            dst = xt_all[:, p, 128 * c:128 * (c + 1)]
                            if p % 2 == 0:
                                nc.vector.tensor_copy(dst, tp[:, 0:128])
                            else:
                                nc.scalar.copy(out=dst, in_=tp[:, 0:128])

                # ---- post-reduce: s, q, A, c0, Psi ----
                # fn0 -= ybar^2 ; s = bn_w / sqrt(fn + eps) ; q = s * W2^T * inv^2
                nc.vector.tensor_mul(t_tmp2, t_statsg[:, 6:7], t_statsg[:, 6:7])
                nc.vector.tensor_sub(t_statsg[:, 0:1], t_statsg[:, 0:1], t_tmp2)
                nc.scalar.activation(out=t_tmp1, in_=t_statsg[:, 0:6],
                                     func=mybir.ActivationFunctionType.Sqrt,
                                     bias=t_eps, scale=1.0)
                nc.vector.reciprocal(t_tmp1, t_tmp1)
                nc.vector.tensor_mul(t_tmp1, t_tmp1, t_bnwt)   # s [v, l]
                nc.vector.tensor_mul(t_tmp1, t_tmp1, t_w2s)    # q [v, l]

                a_ps = pssm.tile([128, 6], F32, tag="T")
                for l in range(6):
                    nc.tensor.matmul(a_ps[:, l:l + 1],
                                     t_w1t[:, 128 * l:128 * (l + 1)],
                                     t_tmp1[:, l:l + 1], start=True, stop=True)
                nc.vector.tensor_copy(t_A, a_ps)

                # c0 = sum_v (bnbw - ybar * q0 * sqrt(128))
                nc.vector.tensor_mul(t_tmp2, t_statsg[:, 6:7], t_tmp1[:, 0:1])
                nc.scalar.mul(out=t_tmp2, in_=t_tmp2, mul=math.sqrt(float(MUL)))
                nc.vector.tensor_sub(t_tmp2, t_bnbw, t_tmp2)
                c0_ps = pssm.tile([1, 1], F32, tag="T")
                nc.tensor.matmul(c0_ps, t_tmp2[:], t_one32[:],
                                 start=True, stop=True)
                nc.vector.tensor_copy(t_c0row[0:1, 0:1], c0_ps)

                # Psi: column i of pair-p tile gets A[:, l(p)] (i == p)
                for p in range(NPAIR):
                    dst = t_psi[:, 36 * p + p:36 * p + p + 1]
                    src = t_A[:, L_OF[p]:L_OF[p] + 1]
                    if p % 2 == 0:
                        nc.vector.tensor_copy(dst, src)
                    else:
                        nc.scalar.copy(out=dst, in_=src)

            # =========== Phase C: z^T ==========
            with tc.tile_pool(name="ps_z", bufs=1, space="PSUM") as psz:
                zt_ps = psz.tile([36, BS], F32)
                for p in range(NPAIR):
                    nc.tensor.matmul(zt_ps, t_psi[:, 36 * p:36 * (p + 1)],
                                     xt_all[:, p, :],
                                     start=(p == 0), stop=False)
                nc.tensor.matmul(zt_ps, t_c0row[:], t_onesrow[:],
                                 start=False, stop=True)
                nc.vector.tensor_copy(t_zts[0:36, :], zt_ps)
                # replicate z^T to partitions 64:100 for the odd-half chunks
                nc.sync.dma_start(out=t_zts[64:100, :], in_=t_zts[0:36, :])

            # =========== Phase D: S2 grid -> sigmoid -> z2 ==========
            with (
                tc.tile_pool(name="ps_grid", bufs=2, space="PSUM") as psgr,
                tc.tile_pool(name="ps_z2", bufs=1, space="PSUM") as psz2,
                tc.tile_pool(name="sg", bufs=2) as sgp,
            ):
                z2_ps = psz2.tile([36, BS], F32)
                groups = [list(range(s, min(s + SIG_GROUP, NGC)))
                          for s in range(0, NGC, SIG_GROUP)]
                for grp in groups:
                    nj = len(grp)
                    gr_ps = psgr.tile([128, SIG_GROUP, BS], F32, tag="grid")
                    sg = sgp.tile([128, SIG_GROUP, BS], F32R, tag="sg")
                    for j, gc in enumerate(grp):
                        h, cblk = gc % 2, gc // 2
                        nc.tensor.matmul(
                            gr_ps[:, j, :],
                            t_yto2[64 * h:64 * h + 36, 128 * cblk:128 * (cblk + 1)],
                            t_zts[64 * h:64 * h + 36, :], start=True, stop=True)
                    nc.scalar.activation(out=sg[:, 0:nj, :], in_=gr_ps[:, 0:nj, :],
                                         func=mybir.ActivationFunctionType.Sigmoid)
                    for j, gc in enumerate(grp):
                        nc.tensor.matmul(z2_ps, t_yfromt[:, 36 * gc:36 * (gc + 1)],
                                         sg[:, j, :],
                                         start=(gc == 0), stop=(gc == NGC - 1))
                nc.vector.tensor_copy(t_z2s[:], z2_ps)

            # =========== Phase E: W3 expansion + output ==========
            # out[b, OFFS_l + v*d + m] = z2[b, HOFF_l + m] * W3[l][v] — a
            # broadcast multiply, done in exact fp32 as tensor_scalar ops
            # (in0 = replicated W3 row block, scalar = z2 column) spread
            # across Vector/Scalar/GpSimd.  Needs z2 as [b, i]: 4 PE
            # transposes of the [36, 128] z2^T chunks.
            with (
                tc.tile_pool(name="ps_zn", bufs=2, space="PSUM") as psn,
                tc.tile_pool(name="osb", bufs=2) as osbp,
            ):
                for c in range(BCHUNKS):
                    zn_ps = psn.tile([128, 36], F32)
                    nc.tensor.transpose(zn_ps, t_z2s[:, 128 * c:128 * (c + 1)],
                                        t_ident[0:36, 0:36])
                    nc.vector.tensor_copy(t_z2n[:, c, :], zn_ps)
                for c in range(BCHUNKS):
                    # pair-major contiguous blocks; the host un-permutes
                    # columns (inverse of the x permutation) after gather.
                    osb = osbp.tile([128, FEAT], F32, tag="osb")
                    for p in range(NPAIR):
                        l = L_OF[p]
                        blk = osb[:, 128 * p:128 * (p + 1)]
                        w3b = t_w3bc[:, 128 * l:128 * (l + 1)]
                        sc = t_z2n[:, c, p:p + 1]
                        eng = (nc.vector, nc.scalar, nc.vector, nc.gpsimd)[p % 4]
                        if eng is nc.scalar:
                            nc.scalar.activation(
                                out=blk, in_=w3b,
                                func=mybir.ActivationFunctionType.Copy,
                                scale=sc)
                        else:
                            eng.tensor_scalar_mul(blk, w3b, sc)
                    lo = 128 * c
                    nc.sync.dma_start(out=out[lo:lo + 128, :], in_=osb[:])

    nc.compile()
    return nc


# ---------------------------------------------------------------------------
# Public entry point
# ---------------------------------------------------------------------------

_CACHE = {}


def _get_nc():
    if "nc" not in _CACHE:
        _CACHE["nc"] = build_nc()
    return _CACHE["nc"]


def make_in_maps(x, W1, W2, W3, bn_w, bn_b):
    consts = _host_constants()
    weights = _host_weights(W1, W2, W3, bn_w, bn_b)
    xp = _permute_x(x)
    shared = {**consts, **weights}
    in_maps = []
    for c in range(NCORES):
        m = dict(shared)
        m["xp"] = np.ascontiguousarray(xp[BS * c:BS * (c + 1)])
        in_maps.append(m)
    return in_maps


def kernel(x, W1, W2, W3, bn_w, bn_b, trace=False, **run_kwargs):
    nc = _get_nc()
    in_maps = make_in_maps(x, W1, W2, W3, bn_w, bn_b)
    res = run_bass_kernel_spmd(nc, in_maps, list(range(NCORES)),
                               trace=trace, **run_kwargs)
    dev = np.concatenate([res.results[c]["out"] for c in range(NCORES)], axis=0)
    out = np.empty_like(dev)
    out[:, _perm()] = dev      # device output is pair-major; undo the permute
    kernel.last_results = res
    return out.astype(np.float32)
